# revision 89
# baseline (speedup 1.0000x reference)
"""Trainium2 Bass kernel for nn_DualAttentionLayer (dense dual-stream
transformer layer: 2x self-attention -> cross-attention -> gated merge ->
FFN, with layernorms).

Sharding: 8 cores = 4 batches x 2 streams. Core c handles batch c//2,
stream c%2 (0=body, 1=limb). Each core redundantly computes BOTH streams'
self-attention+LN stage (so no inter-core communication is needed), then
its own stream's cross-attention, gate, FFN and final norms.

v2: fp8 compute path.
 - All projection / FFN / AV matmuls use fp8e4m3 inputs with DoubleRow
   perf mode (two 128-deep K tiles per pass, 2x row rate).  Weights are
   pre-scaled by 16 on the host so fp8 quantization operates in the
   normal range; the 1/16 factors are folded into eviction scales and
   the softmax exp scale.
 - Scores (q.k^T, K=64 per head) stay bf16.
 - exp() writes fp8e5m2 u = 64*exp(s*score); the 64 cancels in the
   softmax normalization (rowsum trick via a ones-column in V).
 - All partition broadcasts (softmax 1/rowsum, LN mu/rstd, gate) use
   gpsimd partition_broadcast instead of ones-matmuls.
 - LayerNorm interior math runs in bf16 on DVE (2-byte fast modes);
   PSUM evictions and casts are spread across DVE / Pool / ACT.
"""

import math
import numpy as np
from contextlib import ExitStack

import concourse.bacc as bacc
import concourse.bass as bass
import concourse.mybir as mybir
import concourse.tile as tile
from concourse.bass_utils import run_bass_kernel_spmd

dt = mybir.dt
AF = mybir.ActivationFunctionType
ALU = mybir.AluOpType
PM = mybir.MatmulPerfMode
BF16 = dt.np(dt.bfloat16)
F8NP = dt.np(dt.float8e4)

B, S, E, NH, D = 4, 1024, 512, 8, 64
HID = 4 * E
P = 128
KS = E // P          # 4 feature slabs of 128
NT = S // 512        # 2 token n-tiles of 512
MT = S // P          # 8 token m-tiles of 128
HKS = HID // P       # 16 hidden slabs
EPS = 1e-5
WS = 16.0            # host-side fp8 weight scale
C_EXP = 16.0         # softmax exp output scale (cancels in normalization)
LN_C = math.log(C_EXP)
VB = NH * 65 + 8     # v block stride per k-tile, padded to 528:
                     # dual-fp8 Ldweights needs pair stride % 16 == 0

F32 = dt.float32
BF = dt.bfloat16
F8 = dt.float8e4
F8U = dt.float8e5


def _build_nc(scale: float):
    nc = bacc.Bacc("TRN2", target_bir_lowering=False, debug=False,
                   num_devices=8)

    def din(name, shape, dty=F32):
        return nc.dram_tensor(name, shape, dty, kind="ExternalInput").ap()

    # activations (pre-transposed on host, feature-major [E, S])
    xo32 = din("xo32", [E, S])          # own stream input, fp32 (residual)
    xt16 = din("xt16", [E, S], BF)      # other stream input, bf16 (residual)
    xo8 = din("xo8", [E, S], F8)        # own, fp8 (matmul rhs)
    xt8 = din("xt8", [E, S], F8)

    # attention weight sets: a = self-own, b = self-other, c = cross
    # q/k/v fp8 (x16); out-proj bf16 (unscaled) for accuracy
    attw = {}
    for tag in ("a", "b", "c"):
        for m in ("qw", "kw", "vw"):
            attw[tag + m] = din(tag + m, [E, E], F8)
        attw[tag + "ow"] = din(tag + "ow", [E, E], BF)
        attw[tag + "qb"] = din(tag + "qb", [E])     # 16*qb
        attw[tag + "ob"] = din(tag + "ob", [E])     # ob + vb@ow (unscaled)

    f1w = din("f1w", [E, HID], BF)
    f1b = din("f1b", [HID])
    f2w = din("f2w", [HID, E], BF)
    f2b = din("f2b", [E])
    gw = din("gw", [2 * E, 2], F8)
    gbd = din("gbd", [1, 1])            # gate_b[0] - gate_b[1]

    # norm params: a_own, a_oth (post-self-attn), b (post-gate), c (post-ffn)
    nrm = {}
    for tag in ("nao", "nat", "nb", "nc"):
        nrm[tag + "g"] = din(tag + "g", [E])
        nrm[tag + "b"] = din(tag + "b", [E])

    out_t = nc.dram_tensor("outT", [E, S], F32, kind="ExternalOutput").ap()

    with TileKernel(nc, scale) as tk:
        tk.run(xo32, xt16, xo8, xt8, attw, f1w, f1b, f2w, f2b, gw, gbd,
               nrm, out_t)

    nc.finalize()
    return nc


DEBUG_DUMPS = False


class TileKernel:
    def __init__(self, nc, scale):
        self.nc = nc
        self.scale = float(scale)
        self.ctx = ExitStack()
        self.poolid = 0

    attn_idx = 0

    def dump(self, name, ap):
        if not DEBUG_DUMPS:
            return
        d = self.nc.dram_tensor("dbg_" + name, list(ap.shape), ap.dtype,
                                kind="ExternalOutput").ap()
        self.nc.sync.dma_start(d, ap)

    def __enter__(self):
        self.tc = self.ctx.enter_context(tile.TileContext(self.nc))
        return self

    def __exit__(self, *a):
        return self.ctx.__exit__(*a)

    # ---------- helpers ----------

    def load_vec(self, pool, dram_ap, n, name=None):
        """Load a [n*128] fp32 vector as [128, n] (slab per column)."""
        t = pool.tile([P, n], F32, tag=name)
        self.nc.sync.dma_start(
            t[:], dram_ap.rearrange("(s p) -> p s", p=P))
        return t

    def load_w8(self, pool, dram_ap, in_dim, out_dim, name=None, dty=F8):
        """Load weight [in,out] as [128, (in/128)*out] slab-major."""
        ks = in_dim // P
        t = pool.tile([P, ks * out_dim], dty, tag=name)
        self.nc.sync.dma_start(
            t[:].rearrange("p (s o) -> p s o", s=ks),
            dram_ap.rearrange("(s p) o -> p s o", p=P))
        return t

    def proj_bf(self, wsb, rhs16, in_dim, out_dim, evict, bufs=4):
        """bf16 Form-B projection: out^T = W^T @ x^T."""
        nc = self.nc
        self.poolid += 1
        with self.tc.tile_pool(name=f"pb{self.poolid}", bufs=bufs,
                               space="PSUM") as pp:
            nks = in_dim // P
            wr = wsb[:].rearrange("p (s o) -> p s o", s=nks)
            xr = rhs16[:].rearrange("p (s t) -> p s t", s=nks)
            for ms in range(out_dim // P):
                for nt in range(NT):
                    ps = pp.tile([P, 512], F32, tag="proj", name="proj")
                    for k in range(nks):
                        nc.tensor.matmul(
                            ps[:], wr[:, k, ms * P: ms * P + P],
                            xr[:, k, nt * 512: nt * 512 + 512],
                            start=(k == 0), stop=(k == nks - 1))
                    evict(ps, ms, nt)

    def proj_f8(self, wsb, rhs8, in_dim, out_dim, evict, bufs=4):
        """out^T[out,tok] = (W^T @ x^T) with fp8 DoubleRow matmuls.
        evict(ps, ms, nt) consumes a [128,512] fp32 PSUM tile."""
        nc = self.nc
        self.poolid += 1
        with self.tc.tile_pool(name=f"pp{self.poolid}", bufs=bufs,
                               space="PSUM") as pp:
            self._proj_f8(pp, wsb, rhs8, in_dim, out_dim, evict)

    def _proj_f8(self, pp, wsb, rhs8, in_dim, out_dim, evict):
        nc = self.nc
        nks = in_dim // P
        npr = nks // 2
        wr = wsb[:].rearrange("p (s o) -> p s o", s=nks)
        xr = rhs8[:].rearrange("p (s t) -> p s t", s=nks)
        for ms in range(out_dim // P):
            for nt in range(NT):
                ps = pp.tile([P, 512], F32, tag="proj", name="proj")
                for win in range(2):
                    o = ps[:, win * 256: win * 256 + 256]
                    toff = nt * 512 + win * 256
                    for kp in range(npr):
                        nc.tensor.matmul(
                            o,
                            wr[:, 2 * kp: 2 * kp + 2, ms * P: ms * P + P],
                            xr[:, 2 * kp: 2 * kp + 2, toff: toff + 256],
                            start=(kp == 0), stop=(kp == npr - 1),
                            perf_mode=PM.DoubleRow)
                evict(ps, ms, nt)

    # ---------- attention ----------

    def attention_core(self, name, q8, kv8, wq, wk, wv, qb, oup):
        """MHA core: fp8 projections, bf16 scores, fp8 exp, DoubleRow AV,
        streamed softmax normalization.  Returns the normalized per-head
        output o16 (bf16, tile in caller pool `oup`, = 16x true o).
        The caller runs the out-projection separately (attention_finish)
        so the next attention's core can overlap this one's tail."""
        nc, tc = self.nc, self.tc
        ou = oup.tile([P, KS * S], BF, tag="ou", name="ou_" + name,
                      bufs=2)
        with ExitStack() as actx:
            ap = actx.enter_context(
                tc.tile_pool(name="attc_" + name, bufs=1))
            up = actx.enter_context(
                tc.tile_pool(name="attu_" + name, bufs=3))

            qt = ap.tile([P, KS * S], BF, tag="qT")
            kt = ap.tile([P, KS * S], BF, tag="kT")
            vt = ap.tile([P, MT * VB], F8, tag="vT")

            def ev_q(ps, ms, nt):
                nc.vector.tensor_scalar(
                    qt[:, ms * S + nt * 512: ms * S + nt * 512 + 512],
                    ps[:], qb[:, ms: ms + 1], None, op0=ALU.add)

            def ev_k(ps, ms, nt):
                # ACT (idle during projections; GPSIMD cannot read PSUM)
                nc.scalar.copy(
                    kt[:, ms * S + nt * 512: ms * S + nt * 512 + 512], ps[:])

            self.poolid += 1
            with tc.tile_pool(name=f"attn_pp{self.poolid}", bufs=4,
                              space="PSUM") as pp:
                self._proj_f8(pp, wq, q8, E, E, ev_q)
                self._proj_f8(pp, wk, kv8, E, E, ev_k)

                # V: Form A (x^T as lhsT) -> token-major v [tok, feat],
                # strided into per-head 65-wide blocks, col 64 = 1.
                v4 = vt[:].rearrange("p (m c) -> p m c", m=MT)
                nc.gpsimd.memset(
                    v4[:, :, 0:NH * 65]
                    .rearrange("p m (h c) -> p m h c", h=NH)
                    [:, :, :, 64:65], 1.0)
                xr = kv8[:].rearrange("p (s t) -> p s t", s=KS)
                wvr = wv[:].rearrange("p (s o) -> p s o", s=KS)
                for mt in range(MT):
                    ps = pp.tile([P, 512], F32, tag="proj", name="proj")
                    for fw in range(2):
                        o = ps[:, fw * 256: fw * 256 + 256]
                        for kp in range(2):
                            nc.tensor.matmul(
                                o,
                                xr[:, 2 * kp: 2 * kp + 2, mt * P: mt * P + P],
                                wvr[:, 2 * kp: 2 * kp + 2,
                                    fw * 256: fw * 256 + 256],
                                start=(kp == 0), stop=(kp == 1),
                                perf_mode=PM.DoubleRow)
                    nc.scalar.copy(
                        v4[:, mt, 0:NH * 65]
                        .rearrange("p (h c) -> p h c", h=NH)[:, :, 0:64],
                        ps[:].rearrange("p (h d) -> p h d", h=NH))

            self.poolid += 1
            scav = ExitStack()
            sp = scav.enter_context(
                tc.tile_pool(name=f"attn_sc{self.poolid}", bufs=2,
                             space="PSUM"))
            avp = scav.enter_context(
                tc.tile_pool(name=f"attn_av{self.poolid}", bufs=4,
                             space="PSUM"))

            if name == "own":
                self.dump("o_qt", qt[:])
                self.dump("o_kt", kt[:])
                self.dump("o_vt", vt[:])
            v4 = vt[:].rearrange("p (m c) -> p m c", m=MT)
            sexp = self.scale / (WS * WS)
            # head pairs outer, qn inner: each pair's softmax rowsums are
            # reciprocal'd + broadcast + applied as soon as the pair is
            # done, overlapping the remaining pairs' scores/exp/AV.
            for j in range(NH // 2):
                hs = (2 * j, 2 * j + 1)
                rbh = {}
                for qn in range(NT):
                    av = {(h, w): avp.tile([65, 256], F32, tag="av",
                                           name=f"av{h}_{w}")
                          for h in hs for w in range(2)}
                    for wave in range(MT // 2):
                        sc = {h: sp.tile([P, 1024], F32, tag="sc",
                                         name=f"sc{h}") for h in hs}
                        for i in range(2):
                            mt = wave * 2 + i
                            for h in hs:
                                bp = (h % 2) * 64
                                sl = h // 2
                                nc.tensor.matmul(
                                    sc[h][:, i * 512: i * 512 + 512],
                                    kt[bp: bp + 64,
                                       sl * S + mt * P: sl * S + mt * P + P],
                                    qt[bp: bp + 64,
                                       sl * S + qn * 512: sl * S + qn * 512 + 512],
                                    start=True, stop=True)
                        ut = {h: up.tile([P, 1024], F8, tag="u",
                                         name=f"u{h}") for h in hs}
                        for h in hs:
                            nc.scalar.activation(
                                ut[h][:], sc[h][:], AF.Exp, scale=sexp,
                                bias=self.lnc_c[:, 0:1])
                        for h in hs:
                            utr = ut[h][:].rearrange("p (i t) -> p i t", i=2)
                            for win in range(2):
                                nc.tensor.matmul(
                                    av[h, win][:],
                                    v4[:, 2 * wave: 2 * wave + 2,
                                       h * 65: h * 65 + 65],
                                    utr[:, :, win * 256: win * 256 + 256],
                                    start=(wave == 0), stop=(wave == 3),
                                    perf_mode=PM.DoubleRow)
                    for h in hs:
                        bp = (h % 2) * 64
                        sl = h // 2
                        # unnormalized o^T -> SBUF; 1/rowsum -> broadcast
                        qoff = sl * S + qn * 512
                        st = up.tile([1, 512], F32, tag="rstage",
                                     name="rstage")
                        for win in range(2):
                            nc.vector.tensor_copy(
                                ou[bp: bp + 64,
                                   qoff + win * 256: qoff + win * 256 + 256],
                                av[h, win][0:64, :])
                            nc.vector.tensor_copy(
                                st[:, win * 256: win * 256 + 256],
                                av[h, win][64:65, :])
                        rr = up.tile([1, 512], BF, tag="rrec",
                                     name="rrec")
                        with nc.allow_low_precision(
                                reason="1/rowsum to bf16 is plenty"):
                            nc.vector.reciprocal(rr[:], st[:])
                        # full-128 broadcast: HW ucode mishandles
                        # non-zero destination base partitions
                        rbt = ap.tile([P, 512], BF, tag="rbh",
                                      name=f"rbh{h % 2}_{qn}", bufs=6)
                        nc.gpsimd.partition_broadcast(rbt[:], rr[:])
                        rbh[h, qn] = rbt
                for h in hs:
                    bp = (h % 2) * 64
                    sl = h // 2
                    for qn in range(NT):
                        qoff = sl * S + qn * 512
                        nc.vector.tensor_tensor(
                            ou[bp: bp + 64, qoff: qoff + 512],
                            ou[bp: bp + 64, qoff: qoff + 512],
                            rbh[h, qn][bp: bp + 64, :], op=ALU.mult)
                if name == "own" and j == 0:
                    self.dump("o_rb0", rbh[0, 0][:])
            scav.close()
        if name == "own":
            self.dump("o_ou", ou[:])
        return ou

    def attention_finish(self, ou, wo, evict_out):
        """Out projection (bf16) of a finished attention core.  bufs=2
        keeps PSUM pressure low enough to coexist with the next
        attention core's score/AV pools."""
        self.proj_bf(wo, ou, E, E, evict_out, bufs=2)

    # ---------- layernorm ----------

    def layer_norm(self, t32, gam, bet, out32, out8, out16=None,
                   out_dma=None, bf_in=False):
        """LN over features (partition axis) of t32 [128, KS*S].
        Stats come from a bf16 copy (ones-matmul over partitions); the
        normalize path runs in fp32 when out32 is requested (accuracy),
        bf16 otherwise.  Optional fp8 / bf16 side outputs; out_dma
        streams the fp32 output to DRAM per slab.  bf_in: t32 is
        already bf16 (skip the cast)."""
        nc = self.nc
        self.poolid += 1
        with self.tc.tile_pool(name=f"lnsb{self.poolid}", bufs=1) as lnp:
            if bf_in:
                t16 = t32
            else:
                t16 = lnp.tile([P, KS * S], BF, tag="ln_t16")
                for k in range(KS):
                    sl = slice(k * S, k * S + S)
                    nc.gpsimd.tensor_copy(t16[:, sl], t32[:, sl])
            mu = lnp.tile([1, S], F32, tag="ln_mu", name="ln_mu")
            var = lnp.tile([1, S], F32, tag="ln_row", name="ln_var",
                           bufs=2)
            self.poolid += 1
            with self.tc.tile_pool(name=f"lnp{self.poolid}", bufs=2,
                                   space="PSUM") as sp1:
                for nt in range(NT):
                    pmu = sp1.tile([1, 512], F32, tag="ln_stat", name="pmu")
                    psq = sp1.tile([1, 512], F32, tag="ln_stat", name="psq")
                    for k in range(KS):
                        sl = slice(k * S + nt * 512, k * S + nt * 512 + 512)
                        tsq = lnp.tile([P, 512], BF, tag="ln_tsq",
                                       name="ln_tsq", bufs=2)
                        nc.vector.tensor_tensor(tsq[:], t16[:, sl],
                                                t16[:, sl], op=ALU.mult)
                        nc.tensor.matmul(
                            pmu[:], self.ones_mean[:, 0:1], t16[:, sl],
                            start=(k == 0), stop=(k == KS - 1))
                        nc.tensor.matmul(
                            psq[:], self.ones_mean[:, 0:1], tsq[:],
                            start=(k == 0), stop=(k == KS - 1))
                    osl = slice(nt * 512, nt * 512 + 512)
                    nc.vector.tensor_copy(mu[:, osl], pmu[:])
                    mu2 = lnp.tile([1, 512], F32, tag="ln_mu2", name="ln_mu2")
                    nc.vector.tensor_tensor(mu2[:], mu[:, osl], mu[:, osl],
                                            op=ALU.mult)
                    nc.vector.tensor_tensor(var[:, osl], psq[:], mu2[:],
                                            op=ALU.subtract)
            # rstd = exp(-0.5*ln(var+eps)) (tiny rows)
            lnv = lnp.tile([1, S], F32, tag="ln_row", name="ln_lnv",
                           bufs=2)
            nc.scalar.activation(lnv[:], var[:], AF.Ln,
                                 bias=self.eps_c[:, 0:1])
            rstd = lnp.tile([1, S], F32, tag="ln_row", name="ln_rstd",
                            bufs=2)
            nc.scalar.activation(rstd[:], lnv[:], AF.Exp, scale=-0.5)
            fp32_path = out32 is not None
            bdt = F32 if fp32_path else BF
            if fp32_path:
                murow, rsrow = mu, rstd
            else:
                murow = lnp.tile([1, S], BF, tag="ln_mu16")
                nc.vector.tensor_copy(murow[:], mu[:])
                rsrow = lnp.tile([1, S], BF, tag="ln_rstd16")
                nc.vector.tensor_copy(rsrow[:], rstd[:])
            mub = lnp.tile([P, S], bdt, tag="ln_mub")
            rstdb = lnp.tile([P, S], bdt, tag="ln_rstdb")
            nc.gpsimd.partition_broadcast(mub[:], murow[:])
            nc.gpsimd.partition_broadcast(rstdb[:], rsrow[:])
            src = t32 if fp32_path else t16
            for k in range(KS):
                for nh in range(NT):
                    sl = slice(k * S + nh * 512, k * S + nh * 512 + 512)
                    bsl = slice(nh * 512, nh * 512 + 512)
                    w = lnp.tile([P, 512], bdt, tag="ln_w", name="ln_w",
                                 bufs=2)
                    nc.vector.tensor_tensor(w[:], src[:, sl], mub[:, bsl],
                                            op=ALU.subtract)
                    nc.vector.tensor_tensor(w[:], w[:], rstdb[:, bsl],
                                            op=ALU.mult)
                    if out32 is not None:
                        nc.vector.tensor_scalar(
                            out32[:, sl], w[:], gam[:, k: k + 1],
                            bet[:, k: k + 1], op0=ALU.mult, op1=ALU.add)
                    if out16 is not None:
                        nc.gpsimd.tensor_scalar(
                            out16[:, sl], w[:], gam[:, k: k + 1],
                            bet[:, k: k + 1], op0=ALU.mult, op1=ALU.add)
                    if out8 is not None:
                        eng = nc.gpsimd if out16 is None else nc.vector
                        eng.tensor_scalar(
                            out8[:, sl], w[:], gam[:, k: k + 1],
                            bet[:, k: k + 1], op0=ALU.mult, op1=ALU.add)
                if out32 is not None and out_dma is not None:
                    nc.sync.dma_start(
                        out_dma.rearrange("(s p) t -> p s t", p=P)
                        [:, k, :],
                        out32[:, k * S: k * S + S])

    # ---------- main ----------

    def run(self, xo32, xt16, xo8, xt8, attw, f1w, f1b, f2w, f2b, gw, gbd,
            nrm, out_t):
        nc, tc, ctx = self.nc, self.tc, self.ctx

        const = ctx.enter_context(tc.tile_pool(name="const", bufs=1))

        self.ones_mean = const.tile([P, 1], BF)
        nc.vector.memset(self.ones_mean[:], 1.0 / E)
        self.eps_c = const.tile([1, 1], F32)
        nc.vector.memset(self.eps_c[:], EPS)
        self.lnc_c = const.tile([P, 1], F32)
        nc.vector.memset(self.lnc_c[:], LN_C)
        self.gbdneg = const.tile([1, 1], F32)
        nc.sync.dma_start(self.gbdneg[:], gbd[:])
        nc.vector.tensor_scalar(self.gbdneg[:], self.gbdneg[:], -1.0, None,
                                op0=ALU.mult)

        gam = {t: self.load_vec(const, nrm[t + "g"], KS, name=t + "g")
               for t in ("nao", "nat", "nb", "nc")}
        bet = {t: self.load_vec(const, nrm[t + "b"], KS, name=t + "b")
               for t in ("nao", "nat", "nb", "nc")}
        # ---- weight prefetch: set 'a' first, then the stage-1 inputs
        # (unblocking the first projections ASAP), then the rest ----
        wp = ctx.enter_context(tc.tile_pool(name="wp_all", bufs=1))
        act = ctx.enter_context(tc.tile_pool(name="acts", bufs=1))
        oup = ctx.enter_context(tc.tile_pool(name="oup", bufs=1))

        W = {}

        def load_set(tag):
            for m in ("qw", "kw", "vw"):
                W[tag + m] = self.load_w8(wp, attw[tag + m], E, E, tag + m)
            W[tag + "ow"] = self.load_w8(wp, attw[tag + "ow"], E, E,
                                         tag + "ow", dty=BF)
            W[tag + "qb"] = self.load_vec(wp, attw[tag + "qb"], KS,
                                          tag + "qb")
            W[tag + "ob"] = self.load_vec(wp, attw[tag + "ob"], KS,
                                          tag + "ob")

        load_set("a")

        # ---- stage 1: self-attention + LN for both streams ----
        # Emission order: own.core, oth.core, own.finish, oth.finish —
        # the oth core's ACT-bound exp phase overlaps own's DVE/Pool
        # finish (out-proj evictions + LN).
        s1 = ExitStack()
        pools = {st: s1.enter_context(tc.tile_pool(name="sb_" + st,
                                                   bufs=1))
                 for st in ("own", "oth")}
        s1x = ExitStack()
        x8p = s1x.enter_context(tc.tile_pool(name="s1x", bufs=1))
        xin = {}
        for st, (x32d, x8d) in (("own", (xo32, xo8)),
                                ("oth", (xt16, xt8))):
            sbp = pools[st]
            x8 = x8p.tile([P, KS * S], F8, tag="x8", name="x8" + st,
                          bufs=2)
            nc.sync.dma_start(
                x8[:].rearrange("p (s t) -> p s t", s=KS),
                x8d.rearrange("(s p) t -> p s t", p=P))
            xdt = F32 if st == "own" else BF
            x32 = sbp.tile([P, KS * S], xdt, tag="x32", name="x32")
            nc.sync.dma_start(
                x32[:].rearrange("p (s t) -> p s t", s=KS),
                x32d.rearrange("(s p) t -> p s t", p=P))
            xin[st] = (sbp, x8, x32)

        load_set("b")
        load_set("c")
        gw_sb = wp.tile([P, 8 * 2], F8, tag="gw")
        nc.sync.dma_start(
            gw_sb[:].rearrange("p (s o) -> p s o", s=8),
            gw.rearrange("(s p) o -> p s o", p=P))

        ou1 = {}
        for st, wtag in (("own", "a"), ("oth", "b")):
            sbp, x8, x32 = xin[st]
            ou1[st] = self.attention_core(
                st, x8, x8, W[wtag + "qw"], W[wtag + "kw"],
                W[wtag + "vw"], W[wtag + "qb"], oup)
        s1x.close()

        y32 = None
        y8 = {}
        for st, (wtag, ntag) in (("own", ("a", "nao")),
                                 ("oth", ("b", "nat"))):
            sbp, x8, x32 = xin[st]
            ob = W[wtag + "ob"]
            t1 = x32  # residual accumulates in place over the input
            # residual + ob are pre-scaled x16 on the host; psum is
            # 16*(o@ow), so t1 = 16*(true t1).  LN is scale-invariant.

            def ev_out(ps, ms, nt, _ob=ob, _t1=t1):
                sl = slice(ms * S + nt * 512, ms * S + nt * 512 + 512)
                nc.vector.scalar_tensor_tensor(
                    _t1[:, sl], ps[:], _ob[:, ms: ms + 1], _t1[:, sl],
                    op0=ALU.add, op1=ALU.add)

            self.attention_finish(ou1[st], W[wtag + "ow"], ev_out)
            if st == "own":
                self.dump("t1own", t1[:])
                y32 = act.tile([P, KS * S], F32, tag="a32",
                               name="yo32", bufs=2)
                y8[st] = act.tile([P, KS * S], F8, tag="a8",
                                  name="yo8", bufs=3)
                self.layer_norm(t1, gam[ntag], bet[ntag], y32, y8[st])
                self.dump("y32", y32[:])
            else:
                y8[st] = act.tile([P, KS * S], F8, tag="a8",
                                  name="yt8", bufs=3)
                self.layer_norm(t1, gam[ntag], bet[ntag], None, y8[st],
                                bf_in=True)
        s1.close()

        # ---- stage 2: cross attention ----
        # FFN weights load here: early enough to overlap, after the
        # stage-1 SBUF peak has passed.
        wpf = ctx.enter_context(tc.tile_pool(name="wp_ffn", bufs=1))
        w1 = self.load_w8(wpf, f1w, E, HID, "w1", dty=BF)
        b1 = self.load_vec(wpf, f1b, HKS, "b1")
        w2 = self.load_w8(wpf, f2w, HID, E, "w2", dty=BF)
        b2 = self.load_vec(wpf, f2b, KS, "b2")

        cross32 = act.tile([P, KS * S], F32, tag="a32", bufs=2)
        cross8 = act.tile([P, KS * S], F8, tag="a8", bufs=3)
        with ExitStack() as sctx:
            sbp = sctx.enter_context(tc.tile_pool(name="sb_c", bufs=1))
            ob = W["cob"]

            ouc = self.attention_core(
                "cross", y8["own"], y8["oth"], W["cqw"], W["ckw"],
                W["cvw"], W["cqb"], oup)

            def ev_cross(ps, ms, nt, _ob=ob):
                sl = slice(ms * S + nt * 512, ms * S + nt * 512 + 512)
                nc.vector.tensor_scalar(
                    cross32[:, sl], ps[:], 1.0 / WS,
                    _ob[:, ms: ms + 1], op0=ALU.mult, op1=ALU.add)
                nc.gpsimd.tensor_copy(cross8[:, sl], cross32[:, sl])

            self.attention_finish(ouc, W["cow"], ev_cross)
            self.dump("cross32", cross32[:])

        # ---- stage 3: gate + merge + LN_b ----
        with ExitStack() as sctx:
            sbp = sctx.enter_context(tc.tile_pool(name="sb_g", bufs=1))
            g0row = sbp.tile([1, S], F32, tag="g0")
            gwr = gw_sb[:].rearrange("p (s o) -> p s o", s=8)
            self.poolid += 1
            gp = sctx.enter_context(tc.tile_pool(
                name=f"gp{self.poolid}", bufs=2, space="PSUM"))
            srcs = (y8["own"], cross8)
            for nt in range(NT):
                l0 = gp.tile([1, 512], F32, tag="gl", name="gl0")
                l1 = gp.tile([1, 512], F32, tag="gl", name="gl1")
                for s in range(8):  # 8 gw slabs: 0-3 own, 4-7 cross
                    src = srcs[s // 4]
                    xr = src[:].rearrange("p (s t) -> p s t", s=KS)
                    for col, l in ((0, l0), (1, l1)):
                        nc.tensor.matmul(
                            l[:], gwr[:, s, col: col + 1],
                            xr[:, s % 4, nt * 512: nt * 512 + 512],
                            start=(s == 0), stop=(s == 7))
                l0s = sbp.tile([1, 512], F32, tag="gl0s", name="gl0s")
                nc.scalar.copy(l0s[:], l0[:])
                d = sbp.tile([1, 512], F32, tag="gd", name="gd")
                nc.vector.tensor_tensor(d[:], l1[:], l0s[:],
                                        op=ALU.subtract)
                # g0 = sigmoid(l0-l1+gbd) = 1/(1+exp(l1-l0-gbd))
                eneg = sbp.tile([1, 512], F32, tag="ge", name="ge")
                nc.scalar.activation(eneg[:], d[:], AF.Exp,
                                     scale=1.0 / WS,
                                     bias=self.gbdneg[:, 0:1])
                den = sbp.tile([1, 512], F32, tag="gden", name="gden")
                nc.vector.tensor_scalar(den[:], eneg[:], 1.0, None,
                                        op0=ALU.add)
                nc.vector.reciprocal(
                    g0row[:, nt * 512: nt * 512 + 512], den[:])
            g0b = sbp.tile([P, S], F32, tag="g0b")
            nc.gpsimd.partition_broadcast(g0b[:], g0row[:])
            t2 = sbp.tile([P, KS * S], F32, tag="t2")
            for k in range(KS):
                sl = slice(k * S, k * S + S)
                w = sbp.tile([P, S], F32, tag="gs", name="gs", bufs=2)
                nc.vector.tensor_tensor(w[:], y32[:, sl],
                                        cross32[:, sl], op=ALU.subtract)
                nc.vector.tensor_tensor(w[:], w[:], g0b[:], op=ALU.mult)
                nc.vector.tensor_tensor(t2[:, sl], w[:], cross32[:, sl],
                                        op=ALU.add)
            self.dump("g0row", g0row[:])
            self.dump("t2", t2[:])
            z32 = act.tile([P, KS * S], F32, tag="a32", bufs=2)
            z16 = act.tile([P, KS * S], BF, tag="a16", bufs=1)
            self.layer_norm(t2, gam["nb"], bet["nb"], z32, None,
                            out16=z16)
            self.dump("z32", z32[:])

        # ---- stage 4: FFN (bf16) + LN_c + output ----
        with ExitStack() as sctx:
            sbp = sctx.enter_context(tc.tile_pool(name="sb_f", bufs=1))
            t3 = z32  # FFN residual accumulates in place over z32
            with ExitStack() as fctx:
                hp = fctx.enter_context(tc.tile_pool(name="hp_f", bufs=1))
                h16 = hp.tile([P, HKS * S], BF, tag="h16")

                def ev_gelu(ps, ms, nt):
                    nc.scalar.activation(
                        h16[:, ms * S + nt * 512: ms * S + nt * 512 + 512],
                        ps[:], AF.Gelu, bias=b1[:, ms: ms + 1])

                self.proj_bf(w1, z16, E, HID, ev_gelu)

                def ev_f2(ps, ms, nt):
                    sl = slice(ms * S + nt * 512, ms * S + nt * 512 + 512)
                    nc.vector.scalar_tensor_tensor(
                        t3[:, sl], ps[:], b2[:, ms: ms + 1], z32[:, sl],
                        op0=ALU.add, op1=ALU.add)

                self.proj_bf(w2, h16, HID, E, ev_f2)

            out32 = sbp.tile([P, KS * S], F32, tag="out32")
            self.layer_norm(t3, gam["nc"], bet["nc"], out32, None,
                            out_dma=out_t)


_NC_CACHE = {}


def _get_nc(scale):
    key = round(float(scale), 12)
    if key not in _NC_CACHE:
        _NC_CACHE[key] = _build_nc(scale)
    return _NC_CACHE[key]


def _prep_in_maps(inputs):
    """Slice/transform the full inputs into 8 per-core input dicts."""
    f32 = np.float32
    body = np.asarray(inputs["body_feats"], f32)
    limb = np.asarray(inputs["limb_feats"], f32)
    qw = np.asarray(inputs["attn_qw"], f32)
    qb = np.asarray(inputs["attn_qb"], f32)
    kw = np.asarray(inputs["attn_kw"], f32)
    vw = np.asarray(inputs["attn_vw"], f32)
    vb = np.asarray(inputs["attn_vb"], f32)
    ow = np.asarray(inputs["attn_ow"], f32)
    ob = np.asarray(inputs["attn_ob"], f32)
    f1w = np.asarray(inputs["ffn_w1"], f32)
    f1b = np.asarray(inputs["ffn_b1"], f32)
    f2w = np.asarray(inputs["ffn_w2"], f32)
    f2b = np.asarray(inputs["ffn_b2"], f32)
    ns = np.asarray(inputs["norm_scale"], f32)
    nb = np.asarray(inputs["norm_bias"], f32)
    gw = np.asarray(inputs["gate_w"], f32)
    gb = np.asarray(inputs["gate_b"], f32)

    feats = [body, limb]
    ob_eff = [ob[i] + vb[i] @ ow[i] for i in range(4)]
    gbd = np.array([[gb[0] - gb[1]]], f32)
    ln_a = [0, 3]
    ln_c = [2, 5]

    in_maps = []
    for c in range(8):
        b, s = c // 2, c % 2
        o = s          # own stream / self-attn set
        t = 1 - s      # other stream
        cr = 2 + s     # cross-attn set
        xoT = np.ascontiguousarray(feats[o][b].T)
        xtT = np.ascontiguousarray(feats[t][b].T)
        m = {
            # residual streams pre-scaled x16 (the stage-1 evict adds
            # them to 16x psums; LN is scale-invariant)
            "xo32": WS * xoT,
            "xt16": (WS * xtT).astype(BF16),
            "xo8": xoT.astype(F8NP),
            "xt8": xtT.astype(F8NP),
            "f1w": f1w[s].astype(BF16), "f1b": f1b[s],
            "f2w": f2w[s].astype(BF16), "f2b": f2b[s],
            "gw": (WS * gw).astype(F8NP), "gbd": gbd,
            "naog": ns[ln_a[o]], "naob": nb[ln_a[o]],
            "natg": ns[ln_a[t]], "natb": nb[ln_a[t]],
            "nbg": ns[1], "nbb": nb[1],
            "ncg": ns[ln_c[s]], "ncb": nb[ln_c[s]],
        }
        for tag, i in (("a", o), ("b", t), ("c", cr)):
            m[tag + "qw"] = (WS * qw[i]).astype(F8NP)
            m[tag + "kw"] = (WS * kw[i]).astype(F8NP)
            m[tag + "vw"] = (WS * vw[i]).astype(F8NP)
            m[tag + "ow"] = ow[i].astype(BF16)
            m[tag + "qb"] = WS * qb[i]
            # self-attn evicts add ob to a 16x psum; cross runs at 1x
            m[tag + "ob"] = (WS if tag != "c" else 1.0) * ob_eff[i]
        in_maps.append(m)
    return in_maps


def kernel(**inputs):
    temp = float(np.asarray(inputs["temperature"]))
    scale = (D ** -0.5) / temp
    nc = _get_nc(scale)
    in_maps = _prep_in_maps(inputs)
    res = run_bass_kernel_spmd(nc, in_maps, core_ids=list(range(8)))
    body = np.empty((B, S, E), np.float32)
    limb = np.empty((B, S, E), np.float32)
    for c in range(8):
        b, s = c // 2, c % 2
        o = res.results[c]["outT"].T
        (body if s == 0 else limb)[b] = o
    return body, limb


# revision 91
# speedup vs baseline: 1.0121x; 1.0121x over previous
"""Trainium2 Bass kernel for nn_DualAttentionLayer (dense dual-stream
transformer layer: 2x self-attention -> cross-attention -> gated merge ->
FFN, with layernorms).

Sharding: 8 cores = 4 batches x 2 streams. Core c handles batch c//2,
stream c%2 (0=body, 1=limb). Each core redundantly computes BOTH streams'
self-attention+LN stage (so no inter-core communication is needed), then
its own stream's cross-attention, gate, FFN and final norms.

v2: fp8 compute path.
 - All projection / FFN / AV matmuls use fp8e4m3 inputs with DoubleRow
   perf mode (two 128-deep K tiles per pass, 2x row rate).  Weights are
   pre-scaled by 16 on the host so fp8 quantization operates in the
   normal range; the 1/16 factors are folded into eviction scales and
   the softmax exp scale.
 - Scores (q.k^T, K=64 per head) stay bf16.
 - exp() writes fp8e5m2 u = 64*exp(s*score); the 64 cancels in the
   softmax normalization (rowsum trick via a ones-column in V).
 - All partition broadcasts (softmax 1/rowsum, LN mu/rstd, gate) use
   gpsimd partition_broadcast instead of ones-matmuls.
 - LayerNorm interior math runs in bf16 on DVE (2-byte fast modes);
   PSUM evictions and casts are spread across DVE / Pool / ACT.
"""

import math
import numpy as np
from contextlib import ExitStack

import concourse.bacc as bacc
import concourse.bass as bass
import concourse.mybir as mybir
import concourse.tile as tile
from concourse.bass_utils import run_bass_kernel_spmd

dt = mybir.dt
AF = mybir.ActivationFunctionType
ALU = mybir.AluOpType
PM = mybir.MatmulPerfMode
BF16 = dt.np(dt.bfloat16)
F8NP = dt.np(dt.float8e4)

B, S, E, NH, D = 4, 1024, 512, 8, 64
HID = 4 * E
P = 128
KS = E // P          # 4 feature slabs of 128
NT = S // 512        # 2 token n-tiles of 512
MT = S // P          # 8 token m-tiles of 128
HKS = HID // P       # 16 hidden slabs
EPS = 1e-5
WS = 16.0            # host-side fp8 weight scale
C_EXP = 16.0         # softmax exp output scale (cancels in normalization)
LN_C = math.log(C_EXP)
VB = NH * 65 + 8     # v block stride per k-tile, padded to 528:
                     # dual-fp8 Ldweights needs pair stride % 16 == 0

F32 = dt.float32
BF = dt.bfloat16
F8 = dt.float8e4
F8U = dt.float8e5


def _build_nc(scale: float):
    nc = bacc.Bacc("TRN2", target_bir_lowering=False, debug=False,
                   num_devices=8)

    def din(name, shape, dty=F32):
        return nc.dram_tensor(name, shape, dty, kind="ExternalInput").ap()

    # activations (pre-transposed on host, feature-major [E, S])
    xo32 = din("xo32", [E, S])          # own stream input, fp32 (residual)
    xt16 = din("xt16", [E, S], BF)      # other stream input, bf16 (residual)
    xo8 = din("xo8", [E, S], F8)        # own, fp8 (matmul rhs)
    xt8 = din("xt8", [E, S], F8)

    # attention weight sets: a = self-own, b = self-other, c = cross
    # q/k/v fp8 (x16); out-proj bf16 (unscaled) for accuracy
    attw = {}
    for tag in ("a", "b", "c"):
        for m in ("qw", "kw", "vw"):
            attw[tag + m] = din(tag + m, [E, E], F8)
        attw[tag + "ow"] = din(tag + "ow", [E, E], BF)
        attw[tag + "qb"] = din(tag + "qb", [E])     # 16*qb
        attw[tag + "ob"] = din(tag + "ob", [E])     # ob + vb@ow (unscaled)

    f1w = din("f1w", [E, HID], BF)
    f1b = din("f1b", [HID])
    f2w = din("f2w", [HID, E], BF)
    f2b = din("f2b", [E])
    gw = din("gw", [2 * E, 2], F8)
    gbd = din("gbd", [1, 1])            # gate_b[0] - gate_b[1]

    # norm params: a_own, a_oth (post-self-attn), b (post-gate), c (post-ffn)
    nrm = {}
    for tag in ("nao", "nat", "nb", "nc"):
        nrm[tag + "g"] = din(tag + "g", [E])
        nrm[tag + "b"] = din(tag + "b", [E])

    out_t = nc.dram_tensor("outT", [E, S], F32, kind="ExternalOutput").ap()

    with TileKernel(nc, scale) as tk:
        tk.run(xo32, xt16, xo8, xt8, attw, f1w, f1b, f2w, f2b, gw, gbd,
               nrm, out_t)

    nc.finalize()
    return nc


DEBUG_DUMPS = False


class TileKernel:
    def __init__(self, nc, scale):
        self.nc = nc
        self.scale = float(scale)
        self.ctx = ExitStack()
        self.poolid = 0

    attn_idx = 0

    def dump(self, name, ap):
        if not DEBUG_DUMPS:
            return
        d = self.nc.dram_tensor("dbg_" + name, list(ap.shape), ap.dtype,
                                kind="ExternalOutput").ap()
        self.nc.sync.dma_start(d, ap)

    def __enter__(self):
        self.tc = self.ctx.enter_context(tile.TileContext(self.nc))
        return self

    def __exit__(self, *a):
        return self.ctx.__exit__(*a)

    # ---------- helpers ----------

    def load_vec(self, pool, dram_ap, n, name=None):
        """Load a [n*128] fp32 vector as [128, n] (slab per column)."""
        t = pool.tile([P, n], F32, tag=name)
        self.nc.sync.dma_start(
            t[:], dram_ap.rearrange("(s p) -> p s", p=P))
        return t

    def load_w8(self, pool, dram_ap, in_dim, out_dim, name=None, dty=F8):
        """Load weight [in,out] as [128, (in/128)*out] slab-major."""
        ks = in_dim // P
        t = pool.tile([P, ks * out_dim], dty, tag=name)
        self.nc.sync.dma_start(
            t[:].rearrange("p (s o) -> p s o", s=ks),
            dram_ap.rearrange("(s p) o -> p s o", p=P))
        return t

    def proj_bf(self, wsb, rhs16, in_dim, out_dim, evict, bufs=4):
        """bf16 Form-B projection: out^T = W^T @ x^T."""
        nc = self.nc
        self.poolid += 1
        with self.tc.tile_pool(name=f"pb{self.poolid}", bufs=bufs,
                               space="PSUM") as pp:
            nks = in_dim // P
            wr = wsb[:].rearrange("p (s o) -> p s o", s=nks)
            xr = rhs16[:].rearrange("p (s t) -> p s t", s=nks)
            for ms in range(out_dim // P):
                for nt in range(NT):
                    ps = pp.tile([P, 512], F32, tag="proj", name="proj")
                    for k in range(nks):
                        nc.tensor.matmul(
                            ps[:], wr[:, k, ms * P: ms * P + P],
                            xr[:, k, nt * 512: nt * 512 + 512],
                            start=(k == 0), stop=(k == nks - 1))
                    evict(ps, ms, nt)

    def proj_f8(self, wsb, rhs8, in_dim, out_dim, evict, bufs=4):
        """out^T[out,tok] = (W^T @ x^T) with fp8 DoubleRow matmuls.
        evict(ps, ms, nt) consumes a [128,512] fp32 PSUM tile."""
        nc = self.nc
        self.poolid += 1
        with self.tc.tile_pool(name=f"pp{self.poolid}", bufs=bufs,
                               space="PSUM") as pp:
            self._proj_f8(pp, wsb, rhs8, in_dim, out_dim, evict)

    def _proj_f8(self, pp, wsb, rhs8, in_dim, out_dim, evict):
        nc = self.nc
        nks = in_dim // P
        npr = nks // 2
        wr = wsb[:].rearrange("p (s o) -> p s o", s=nks)
        xr = rhs8[:].rearrange("p (s t) -> p s t", s=nks)
        for ms in range(out_dim // P):
            for nt in range(NT):
                ps = pp.tile([P, 512], F32, tag="proj", name="proj")
                for win in range(2):
                    o = ps[:, win * 256: win * 256 + 256]
                    toff = nt * 512 + win * 256
                    for kp in range(npr):
                        nc.tensor.matmul(
                            o,
                            wr[:, 2 * kp: 2 * kp + 2, ms * P: ms * P + P],
                            xr[:, 2 * kp: 2 * kp + 2, toff: toff + 256],
                            start=(kp == 0), stop=(kp == npr - 1),
                            perf_mode=PM.DoubleRow)
                evict(ps, ms, nt)

    # ---------- attention ----------

    def attention_core(self, name, q8, kv8, wq, wk, wv, qb, oup):
        """MHA core: fp8 projections, bf16 scores, fp8 exp, DoubleRow AV,
        streamed softmax normalization.  Returns the normalized per-head
        output o16 (bf16, tile in caller pool `oup`, = 16x true o).
        The caller runs the out-projection separately (attention_finish)
        so the next attention's core can overlap this one's tail."""
        nc, tc = self.nc, self.tc
        ou = oup.tile([P, KS * S], BF, tag="ou", name="ou_" + name,
                      bufs=2)
        with ExitStack() as actx:
            ap = actx.enter_context(
                tc.tile_pool(name="attc_" + name, bufs=1))
            up = actx.enter_context(
                tc.tile_pool(name="attu_" + name, bufs=3))

            qt = ap.tile([P, KS * S], BF, tag="qT")
            kt = ap.tile([P, KS * S], BF, tag="kT")
            vt = ap.tile([P, MT * VB], F8, tag="vT")

            def ev_q(ps, ms, nt):
                nc.vector.tensor_scalar(
                    qt[:, ms * S + nt * 512: ms * S + nt * 512 + 512],
                    ps[:], qb[:, ms: ms + 1], None, op0=ALU.add)

            def ev_k(ps, ms, nt):
                # ACT (idle during projections; GPSIMD cannot read PSUM)
                nc.scalar.copy(
                    kt[:, ms * S + nt * 512: ms * S + nt * 512 + 512], ps[:])

            self.poolid += 1
            with tc.tile_pool(name=f"attn_pp{self.poolid}", bufs=4,
                              space="PSUM") as pp:
                self._proj_f8(pp, wq, q8, E, E, ev_q)
                self._proj_f8(pp, wk, kv8, E, E, ev_k)

                # V: Form A (x^T as lhsT) -> token-major v [tok, feat],
                # strided into per-head 65-wide blocks, col 64 = 1.
                v4 = vt[:].rearrange("p (m c) -> p m c", m=MT)
                nc.gpsimd.memset(
                    v4[:, :, 0:NH * 65]
                    .rearrange("p m (h c) -> p m h c", h=NH)
                    [:, :, :, 64:65], 1.0)
                xr = kv8[:].rearrange("p (s t) -> p s t", s=KS)
                wvr = wv[:].rearrange("p (s o) -> p s o", s=KS)
                for mt in range(MT):
                    ps = pp.tile([P, 512], F32, tag="proj", name="proj")
                    for fw in range(2):
                        o = ps[:, fw * 256: fw * 256 + 256]
                        for kp in range(2):
                            nc.tensor.matmul(
                                o,
                                xr[:, 2 * kp: 2 * kp + 2, mt * P: mt * P + P],
                                wvr[:, 2 * kp: 2 * kp + 2,
                                    fw * 256: fw * 256 + 256],
                                start=(kp == 0), stop=(kp == 1),
                                perf_mode=PM.DoubleRow)
                    nc.scalar.copy(
                        v4[:, mt, 0:NH * 65]
                        .rearrange("p (h c) -> p h c", h=NH)[:, :, 0:64],
                        ps[:].rearrange("p (h d) -> p h d", h=NH))

            self.poolid += 1
            scav = ExitStack()
            sp = scav.enter_context(
                tc.tile_pool(name=f"attn_sc{self.poolid}", bufs=2,
                             space="PSUM"))
            avp = scav.enter_context(
                tc.tile_pool(name=f"attn_av{self.poolid}", bufs=4,
                             space="PSUM"))

            if name == "own":
                self.dump("o_qt", qt[:])
                self.dump("o_kt", kt[:])
                self.dump("o_vt", vt[:])
            v4 = vt[:].rearrange("p (m c) -> p m c", m=MT)
            sexp = self.scale / (WS * WS)
            # head pairs outer, qn inner: each pair's softmax rowsums are
            # reciprocal'd + broadcast + applied as soon as the pair is
            # done, overlapping the remaining pairs' scores/exp/AV.
            for j in range(NH // 2):
                hs = (2 * j, 2 * j + 1)
                rbh = {}
                for qn in range(NT):
                    av = {(h, w): avp.tile([65, 256], F32, tag="av",
                                           name=f"av{h}_{w}")
                          for h in hs for w in range(2)}
                    for wave in range(MT // 2):
                        sc = {h: sp.tile([P, 1024], F32, tag="sc",
                                         name=f"sc{h}") for h in hs}
                        for i in range(2):
                            mt = wave * 2 + i
                            for h in hs:
                                bp = (h % 2) * 64
                                sl = h // 2
                                nc.tensor.matmul(
                                    sc[h][:, i * 512: i * 512 + 512],
                                    kt[bp: bp + 64,
                                       sl * S + mt * P: sl * S + mt * P + P],
                                    qt[bp: bp + 64,
                                       sl * S + qn * 512: sl * S + qn * 512 + 512],
                                    start=True, stop=True)
                        ut = {h: up.tile([P, 1024], F8, tag="u",
                                         name=f"u{h}") for h in hs}
                        for h in hs:
                            nc.scalar.activation(
                                ut[h][:], sc[h][:], AF.Exp, scale=sexp,
                                bias=self.lnc_c[:, 0:1])
                        for h in hs:
                            utr = ut[h][:].rearrange("p (i t) -> p i t", i=2)
                            for win in range(2):
                                nc.tensor.matmul(
                                    av[h, win][:],
                                    v4[:, 2 * wave: 2 * wave + 2,
                                       h * 65: h * 65 + 65],
                                    utr[:, :, win * 256: win * 256 + 256],
                                    start=(wave == 0), stop=(wave == 3),
                                    perf_mode=PM.DoubleRow)
                    for h in hs:
                        bp = (h % 2) * 64
                        sl = h // 2
                        # unnormalized o^T -> SBUF; 1/rowsum -> broadcast
                        qoff = sl * S + qn * 512
                        st = up.tile([1, 512], F32, tag="rstage",
                                     name="rstage")
                        for win in range(2):
                            nc.vector.tensor_copy(
                                ou[bp: bp + 64,
                                   qoff + win * 256: qoff + win * 256 + 256],
                                av[h, win][0:64, :])
                            nc.vector.tensor_copy(
                                st[:, win * 256: win * 256 + 256],
                                av[h, win][64:65, :])
                        rr = up.tile([1, 512], BF, tag="rrec",
                                     name="rrec")
                        with nc.allow_low_precision(
                                reason="1/rowsum to bf16 is plenty"):
                            nc.vector.reciprocal(rr[:], st[:])
                        # full-128 broadcast: HW ucode mishandles
                        # non-zero destination base partitions
                        rbt = ap.tile([P, 512], BF, tag="rbh",
                                      name=f"rbh{h % 2}_{qn}", bufs=6)
                        nc.gpsimd.partition_broadcast(rbt[:], rr[:])
                        rbh[h, qn] = rbt
                for h in hs:
                    bp = (h % 2) * 64
                    sl = h // 2
                    for qn in range(NT):
                        qoff = sl * S + qn * 512
                        nc.vector.tensor_tensor(
                            ou[bp: bp + 64, qoff: qoff + 512],
                            ou[bp: bp + 64, qoff: qoff + 512],
                            rbh[h, qn][bp: bp + 64, :], op=ALU.mult)
                if name == "own" and j == 0:
                    self.dump("o_rb0", rbh[0, 0][:])
            scav.close()
        if name == "own":
            self.dump("o_ou", ou[:])
        return ou

    def attention_finish(self, ou, wo, evict_out):
        """Out projection (bf16) of a finished attention core.  bufs=2
        keeps PSUM pressure low enough to coexist with the next
        attention core's score/AV pools."""
        self.proj_bf(wo, ou, E, E, evict_out, bufs=2)

    # ---------- layernorm ----------

    def layer_norm(self, t32, gam, bet, out32, out8, out16=None,
                   out_dma=None, bf_in=False):
        """LN over features (partition axis) of t32 [128, KS*S].
        Stats come from a bf16 copy (ones-matmul over partitions); the
        normalize path runs in fp32 when out32 is requested (accuracy),
        bf16 otherwise.  Optional fp8 / bf16 side outputs; out_dma
        streams the fp32 output to DRAM per slab.  bf_in: t32 is
        already bf16 (skip the cast)."""
        nc = self.nc
        self.poolid += 1
        with self.tc.tile_pool(name=f"lnsb{self.poolid}", bufs=1) as lnp:
            if bf_in:
                t16 = t32
            else:
                t16 = lnp.tile([P, KS * S], BF, tag="ln_t16")
                for k in range(KS):
                    sl = slice(k * S, k * S + S)
                    nc.gpsimd.tensor_copy(t16[:, sl], t32[:, sl])
            mu = lnp.tile([1, S], F32, tag="ln_mu", name="ln_mu")
            var = lnp.tile([1, S], F32, tag="ln_row", name="ln_var",
                           bufs=2)
            self.poolid += 1
            with self.tc.tile_pool(name=f"lnp{self.poolid}", bufs=2,
                                   space="PSUM") as sp1:
                for nt in range(NT):
                    pmu = sp1.tile([1, 512], F32, tag="ln_stat", name="pmu")
                    psq = sp1.tile([1, 512], F32, tag="ln_stat", name="psq")
                    for k in range(KS):
                        sl = slice(k * S + nt * 512, k * S + nt * 512 + 512)
                        tsq = lnp.tile([P, 512], BF, tag="ln_tsq",
                                       name="ln_tsq", bufs=2)
                        nc.vector.tensor_tensor(tsq[:], t16[:, sl],
                                                t16[:, sl], op=ALU.mult)
                        nc.tensor.matmul(
                            pmu[:], self.ones_mean[:, 0:1], t16[:, sl],
                            start=(k == 0), stop=(k == KS - 1))
                        nc.tensor.matmul(
                            psq[:], self.ones_mean[:, 0:1], tsq[:],
                            start=(k == 0), stop=(k == KS - 1))
                    osl = slice(nt * 512, nt * 512 + 512)
                    nc.vector.tensor_copy(mu[:, osl], pmu[:])
                    mu2 = lnp.tile([1, 512], F32, tag="ln_mu2", name="ln_mu2")
                    nc.vector.tensor_tensor(mu2[:], mu[:, osl], mu[:, osl],
                                            op=ALU.mult)
                    nc.vector.tensor_tensor(var[:, osl], psq[:], mu2[:],
                                            op=ALU.subtract)
            # rstd = exp(-0.5*ln(var+eps)) (tiny rows)
            lnv = lnp.tile([1, S], F32, tag="ln_row", name="ln_lnv",
                           bufs=2)
            nc.scalar.activation(lnv[:], var[:], AF.Ln,
                                 bias=self.eps_c[:, 0:1])
            rstd = lnp.tile([1, S], F32, tag="ln_row", name="ln_rstd",
                            bufs=2)
            nc.scalar.activation(rstd[:], lnv[:], AF.Exp, scale=-0.5)
            fp32_path = out32 is not None
            bdt = F32 if fp32_path else BF
            if fp32_path:
                murow, rsrow = mu, rstd
            else:
                murow = lnp.tile([1, S], BF, tag="ln_mu16")
                nc.vector.tensor_copy(murow[:], mu[:])
                rsrow = lnp.tile([1, S], BF, tag="ln_rstd16")
                nc.vector.tensor_copy(rsrow[:], rstd[:])
            mub = lnp.tile([P, S], bdt, tag="ln_mub")
            rstdb = lnp.tile([P, S], bdt, tag="ln_rstdb")
            nc.gpsimd.partition_broadcast(mub[:], murow[:])
            nc.gpsimd.partition_broadcast(rstdb[:], rsrow[:])
            src = t32 if fp32_path else t16
            for k in range(KS):
                for nh in range(NT):
                    sl = slice(k * S + nh * 512, k * S + nh * 512 + 512)
                    bsl = slice(nh * 512, nh * 512 + 512)
                    w = lnp.tile([P, 512], bdt, tag="ln_w", name="ln_w",
                                 bufs=2)
                    nc.vector.tensor_tensor(w[:], src[:, sl], mub[:, bsl],
                                            op=ALU.subtract)
                    nc.vector.tensor_tensor(w[:], w[:], rstdb[:, bsl],
                                            op=ALU.mult)
                    if out32 is not None:
                        nc.vector.tensor_scalar(
                            out32[:, sl], w[:], gam[:, k: k + 1],
                            bet[:, k: k + 1], op0=ALU.mult, op1=ALU.add)
                    if out16 is not None:
                        nc.gpsimd.tensor_scalar(
                            out16[:, sl], w[:], gam[:, k: k + 1],
                            bet[:, k: k + 1], op0=ALU.mult, op1=ALU.add)
                    if out8 is not None:
                        eng = nc.gpsimd if out16 is None else nc.vector
                        eng.tensor_scalar(
                            out8[:, sl], w[:], gam[:, k: k + 1],
                            bet[:, k: k + 1], op0=ALU.mult, op1=ALU.add)
                if out32 is not None and out_dma is not None:
                    nc.sync.dma_start(
                        out_dma.rearrange("(s p) t -> p s t", p=P)
                        [:, k, :],
                        out32[:, k * S: k * S + S])

    # ---------- main ----------

    def run(self, xo32, xt16, xo8, xt8, attw, f1w, f1b, f2w, f2b, gw, gbd,
            nrm, out_t):
        nc, tc, ctx = self.nc, self.tc, self.ctx

        const = ctx.enter_context(tc.tile_pool(name="const", bufs=1))

        self.ones_mean = const.tile([P, 1], BF)
        nc.vector.memset(self.ones_mean[:], 1.0 / E)
        self.eps_c = const.tile([1, 1], F32)
        nc.vector.memset(self.eps_c[:], EPS)
        self.lnc_c = const.tile([P, 1], F32)
        nc.vector.memset(self.lnc_c[:], LN_C)
        # ---- weight prefetch: set 'a' first, then the stage-1 inputs
        # (unblocking the first projections ASAP), then the rest ----
        wp = ctx.enter_context(tc.tile_pool(name="wp_all", bufs=1))
        act = ctx.enter_context(tc.tile_pool(name="acts", bufs=1))
        oup = ctx.enter_context(tc.tile_pool(name="oup", bufs=1))

        W = {}

        def load_set(tag):
            for m in ("qw", "kw", "vw"):
                W[tag + m] = self.load_w8(wp, attw[tag + m], E, E, tag + m)
            W[tag + "ow"] = self.load_w8(wp, attw[tag + "ow"], E, E,
                                         tag + "ow", dty=BF)
            W[tag + "qb"] = self.load_vec(wp, attw[tag + "qb"], KS,
                                          tag + "qb")
            W[tag + "ob"] = self.load_vec(wp, attw[tag + "ob"], KS,
                                          tag + "ob")

        load_set("a")

        # ---- stage 1: self-attention + LN for both streams ----
        # Emission order: own.core, oth.core, own.finish, oth.finish —
        # the oth core's ACT-bound exp phase overlaps own's DVE/Pool
        # finish (out-proj evictions + LN).
        s1 = ExitStack()
        pools = {st: s1.enter_context(tc.tile_pool(name="sb_" + st,
                                                   bufs=1))
                 for st in ("own", "oth")}
        s1x = ExitStack()
        x8p = s1x.enter_context(tc.tile_pool(name="s1x", bufs=1))
        xin = {}
        for st, (x32d, x8d) in (("own", (xo32, xo8)),
                                ("oth", (xt16, xt8))):
            sbp = pools[st]
            x8 = x8p.tile([P, KS * S], F8, tag="x8", name="x8" + st,
                          bufs=2)
            nc.sync.dma_start(
                x8[:].rearrange("p (s t) -> p s t", s=KS),
                x8d.rearrange("(s p) t -> p s t", p=P))
            xdt = F32 if st == "own" else BF
            x32 = sbp.tile([P, KS * S], xdt, tag="x32", name="x32")
            nc.sync.dma_start(
                x32[:].rearrange("p (s t) -> p s t", s=KS),
                x32d.rearrange("(s p) t -> p s t", p=P))
            xin[st] = (sbp, x8, x32)

        load_set("b")
        load_set("c")
        gw_sb = wp.tile([P, 8 * 2], F8, tag="gw")
        nc.sync.dma_start(
            gw_sb[:].rearrange("p (s o) -> p s o", s=8),
            gw.rearrange("(s p) o -> p s o", p=P))
        # norm params / gate consts aren't needed until the first LN
        # (~150us in) — load them after the startup-critical DMAs
        self.gbdneg = const.tile([1, 1], F32)
        nc.sync.dma_start(self.gbdneg[:], gbd[:])
        nc.vector.tensor_scalar(self.gbdneg[:], self.gbdneg[:], -1.0, None,
                                op0=ALU.mult)
        gam = {t: self.load_vec(const, nrm[t + "g"], KS, name=t + "g")
               for t in ("nao", "nat", "nb", "nc")}
        bet = {t: self.load_vec(const, nrm[t + "b"], KS, name=t + "b")
               for t in ("nao", "nat", "nb", "nc")}

        ou1 = {}
        for st, wtag in (("own", "a"), ("oth", "b")):
            sbp, x8, x32 = xin[st]
            ou1[st] = self.attention_core(
                st, x8, x8, W[wtag + "qw"], W[wtag + "kw"],
                W[wtag + "vw"], W[wtag + "qb"], oup)
        s1x.close()

        y32 = None
        y8 = {}
        for st, (wtag, ntag) in (("own", ("a", "nao")),
                                 ("oth", ("b", "nat"))):
            sbp, x8, x32 = xin[st]
            ob = W[wtag + "ob"]
            t1 = x32  # residual accumulates in place over the input
            # residual + ob are pre-scaled x16 on the host; psum is
            # 16*(o@ow), so t1 = 16*(true t1).  LN is scale-invariant.

            def ev_out(ps, ms, nt, _ob=ob, _t1=t1):
                sl = slice(ms * S + nt * 512, ms * S + nt * 512 + 512)
                nc.vector.scalar_tensor_tensor(
                    _t1[:, sl], ps[:], _ob[:, ms: ms + 1], _t1[:, sl],
                    op0=ALU.add, op1=ALU.add)

            self.attention_finish(ou1[st], W[wtag + "ow"], ev_out)
            if st == "own":
                self.dump("t1own", t1[:])
                y32 = act.tile([P, KS * S], F32, tag="a32",
                               name="yo32", bufs=2)
                y8[st] = act.tile([P, KS * S], F8, tag="a8",
                                  name="yo8", bufs=3)
                self.layer_norm(t1, gam[ntag], bet[ntag], y32, y8[st])
                self.dump("y32", y32[:])
            else:
                y8[st] = act.tile([P, KS * S], F8, tag="a8",
                                  name="yt8", bufs=3)
                self.layer_norm(t1, gam[ntag], bet[ntag], None, y8[st],
                                bf_in=True)
        s1.close()

        # ---- stage 2: cross attention ----
        # FFN weights load here: early enough to overlap, after the
        # stage-1 SBUF peak has passed.
        wpf = ctx.enter_context(tc.tile_pool(name="wp_ffn", bufs=1))
        w1 = self.load_w8(wpf, f1w, E, HID, "w1", dty=BF)
        b1 = self.load_vec(wpf, f1b, HKS, "b1")
        w2 = self.load_w8(wpf, f2w, HID, E, "w2", dty=BF)
        b2 = self.load_vec(wpf, f2b, KS, "b2")

        cross32 = act.tile([P, KS * S], F32, tag="a32", bufs=2)
        cross8 = act.tile([P, KS * S], F8, tag="a8", bufs=3)
        with ExitStack() as sctx:
            sbp = sctx.enter_context(tc.tile_pool(name="sb_c", bufs=1))
            ob = W["cob"]

            ouc = self.attention_core(
                "cross", y8["own"], y8["oth"], W["cqw"], W["ckw"],
                W["cvw"], W["cqb"], oup)

            def ev_cross(ps, ms, nt, _ob=ob):
                sl = slice(ms * S + nt * 512, ms * S + nt * 512 + 512)
                nc.vector.tensor_scalar(
                    cross32[:, sl], ps[:], 1.0 / WS,
                    _ob[:, ms: ms + 1], op0=ALU.mult, op1=ALU.add)
                nc.gpsimd.tensor_copy(cross8[:, sl], cross32[:, sl])

            self.attention_finish(ouc, W["cow"], ev_cross)
            self.dump("cross32", cross32[:])

        # ---- stage 3: gate + merge + LN_b ----
        with ExitStack() as sctx:
            sbp = sctx.enter_context(tc.tile_pool(name="sb_g", bufs=1))
            g0row = sbp.tile([1, S], F32, tag="g0")
            gwr = gw_sb[:].rearrange("p (s o) -> p s o", s=8)
            self.poolid += 1
            gp = sctx.enter_context(tc.tile_pool(
                name=f"gp{self.poolid}", bufs=2, space="PSUM"))
            srcs = (y8["own"], cross8)
            for nt in range(NT):
                l0 = gp.tile([1, 512], F32, tag="gl", name="gl0")
                l1 = gp.tile([1, 512], F32, tag="gl", name="gl1")
                for s in range(8):  # 8 gw slabs: 0-3 own, 4-7 cross
                    src = srcs[s // 4]
                    xr = src[:].rearrange("p (s t) -> p s t", s=KS)
                    for col, l in ((0, l0), (1, l1)):
                        nc.tensor.matmul(
                            l[:], gwr[:, s, col: col + 1],
                            xr[:, s % 4, nt * 512: nt * 512 + 512],
                            start=(s == 0), stop=(s == 7))
                l0s = sbp.tile([1, 512], F32, tag="gl0s", name="gl0s")
                nc.scalar.copy(l0s[:], l0[:])
                d = sbp.tile([1, 512], F32, tag="gd", name="gd")
                nc.vector.tensor_tensor(d[:], l1[:], l0s[:],
                                        op=ALU.subtract)
                # g0 = sigmoid(l0-l1+gbd) = 1/(1+exp(l1-l0-gbd))
                eneg = sbp.tile([1, 512], F32, tag="ge", name="ge")
                nc.scalar.activation(eneg[:], d[:], AF.Exp,
                                     scale=1.0 / WS,
                                     bias=self.gbdneg[:, 0:1])
                den = sbp.tile([1, 512], F32, tag="gden", name="gden")
                nc.vector.tensor_scalar(den[:], eneg[:], 1.0, None,
                                        op0=ALU.add)
                nc.vector.reciprocal(
                    g0row[:, nt * 512: nt * 512 + 512], den[:])
            g0b = sbp.tile([P, S], F32, tag="g0b")
            nc.gpsimd.partition_broadcast(g0b[:], g0row[:])
            t2 = sbp.tile([P, KS * S], F32, tag="t2")
            for k in range(KS):
                sl = slice(k * S, k * S + S)
                w = sbp.tile([P, S], F32, tag="gs", name="gs", bufs=2)
                nc.vector.tensor_tensor(w[:], y32[:, sl],
                                        cross32[:, sl], op=ALU.subtract)
                nc.vector.tensor_tensor(w[:], w[:], g0b[:], op=ALU.mult)
                nc.vector.tensor_tensor(t2[:, sl], w[:], cross32[:, sl],
                                        op=ALU.add)
            self.dump("g0row", g0row[:])
            self.dump("t2", t2[:])
            z32 = act.tile([P, KS * S], F32, tag="a32", bufs=2)
            z16 = act.tile([P, KS * S], BF, tag="a16", bufs=1)
            self.layer_norm(t2, gam["nb"], bet["nb"], z32, None,
                            out16=z16)
            self.dump("z32", z32[:])

        # ---- stage 4: FFN (bf16) + LN_c + output ----
        with ExitStack() as sctx:
            sbp = sctx.enter_context(tc.tile_pool(name="sb_f", bufs=1))
            t3 = z32  # FFN residual accumulates in place over z32
            with ExitStack() as fctx:
                hp = fctx.enter_context(tc.tile_pool(name="hp_f", bufs=1))
                h16 = hp.tile([P, HKS * S], BF, tag="h16")

                def ev_gelu(ps, ms, nt):
                    nc.scalar.activation(
                        h16[:, ms * S + nt * 512: ms * S + nt * 512 + 512],
                        ps[:], AF.Gelu, bias=b1[:, ms: ms + 1])

                self.proj_bf(w1, z16, E, HID, ev_gelu)

                def ev_f2(ps, ms, nt):
                    sl = slice(ms * S + nt * 512, ms * S + nt * 512 + 512)
                    nc.vector.scalar_tensor_tensor(
                        t3[:, sl], ps[:], b2[:, ms: ms + 1], z32[:, sl],
                        op0=ALU.add, op1=ALU.add)

                self.proj_bf(w2, h16, HID, E, ev_f2)

            out32 = sbp.tile([P, KS * S], F32, tag="out32")
            self.layer_norm(t3, gam["nc"], bet["nc"], out32, None,
                            out_dma=out_t)


_NC_CACHE = {}


def _get_nc(scale):
    key = round(float(scale), 12)
    if key not in _NC_CACHE:
        _NC_CACHE[key] = _build_nc(scale)
    return _NC_CACHE[key]


def _prep_in_maps(inputs):
    """Slice/transform the full inputs into 8 per-core input dicts."""
    f32 = np.float32
    body = np.asarray(inputs["body_feats"], f32)
    limb = np.asarray(inputs["limb_feats"], f32)
    qw = np.asarray(inputs["attn_qw"], f32)
    qb = np.asarray(inputs["attn_qb"], f32)
    kw = np.asarray(inputs["attn_kw"], f32)
    vw = np.asarray(inputs["attn_vw"], f32)
    vb = np.asarray(inputs["attn_vb"], f32)
    ow = np.asarray(inputs["attn_ow"], f32)
    ob = np.asarray(inputs["attn_ob"], f32)
    f1w = np.asarray(inputs["ffn_w1"], f32)
    f1b = np.asarray(inputs["ffn_b1"], f32)
    f2w = np.asarray(inputs["ffn_w2"], f32)
    f2b = np.asarray(inputs["ffn_b2"], f32)
    ns = np.asarray(inputs["norm_scale"], f32)
    nb = np.asarray(inputs["norm_bias"], f32)
    gw = np.asarray(inputs["gate_w"], f32)
    gb = np.asarray(inputs["gate_b"], f32)

    feats = [body, limb]
    ob_eff = [ob[i] + vb[i] @ ow[i] for i in range(4)]
    gbd = np.array([[gb[0] - gb[1]]], f32)
    ln_a = [0, 3]
    ln_c = [2, 5]

    in_maps = []
    for c in range(8):
        b, s = c // 2, c % 2
        o = s          # own stream / self-attn set
        t = 1 - s      # other stream
        cr = 2 + s     # cross-attn set
        xoT = np.ascontiguousarray(feats[o][b].T)
        xtT = np.ascontiguousarray(feats[t][b].T)
        m = {
            # residual streams pre-scaled x16 (the stage-1 evict adds
            # them to 16x psums; LN is scale-invariant)
            "xo32": WS * xoT,
            "xt16": (WS * xtT).astype(BF16),
            "xo8": xoT.astype(F8NP),
            "xt8": xtT.astype(F8NP),
            "f1w": f1w[s].astype(BF16), "f1b": f1b[s],
            "f2w": f2w[s].astype(BF16), "f2b": f2b[s],
            "gw": (WS * gw).astype(F8NP), "gbd": gbd,
            "naog": ns[ln_a[o]], "naob": nb[ln_a[o]],
            "natg": ns[ln_a[t]], "natb": nb[ln_a[t]],
            "nbg": ns[1], "nbb": nb[1],
            "ncg": ns[ln_c[s]], "ncb": nb[ln_c[s]],
        }
        for tag, i in (("a", o), ("b", t), ("c", cr)):
            m[tag + "qw"] = (WS * qw[i]).astype(F8NP)
            m[tag + "kw"] = (WS * kw[i]).astype(F8NP)
            m[tag + "vw"] = (WS * vw[i]).astype(F8NP)
            m[tag + "ow"] = ow[i].astype(BF16)
            m[tag + "qb"] = WS * qb[i]
            # self-attn evicts add ob to a 16x psum; cross runs at 1x
            m[tag + "ob"] = (WS if tag != "c" else 1.0) * ob_eff[i]
        in_maps.append(m)
    return in_maps


def kernel(**inputs):
    temp = float(np.asarray(inputs["temperature"]))
    scale = (D ** -0.5) / temp
    nc = _get_nc(scale)
    in_maps = _prep_in_maps(inputs)
    res = run_bass_kernel_spmd(nc, in_maps, core_ids=list(range(8)))
    body = np.empty((B, S, E), np.float32)
    limb = np.empty((B, S, E), np.float32)
    for c in range(8):
        b, s = c // 2, c % 2
        o = res.results[c]["outT"].T
        (body if s == 0 else limb)[b] = o
    return body, limb


# revision 92
# speedup vs baseline: 1.0178x; 1.0056x over previous
"""Trainium2 Bass kernel for nn_DualAttentionLayer (dense dual-stream
transformer layer: 2x self-attention -> cross-attention -> gated merge ->
FFN, with layernorms).

Sharding: 8 cores = 4 batches x 2 streams. Core c handles batch c//2,
stream c%2 (0=body, 1=limb). Each core redundantly computes BOTH streams'
self-attention+LN stage (so no inter-core communication is needed), then
its own stream's cross-attention, gate, FFN and final norms.

v2: fp8 compute path.
 - All projection / FFN / AV matmuls use fp8e4m3 inputs with DoubleRow
   perf mode (two 128-deep K tiles per pass, 2x row rate).  Weights are
   pre-scaled by 16 on the host so fp8 quantization operates in the
   normal range; the 1/16 factors are folded into eviction scales and
   the softmax exp scale.
 - Scores (q.k^T, K=64 per head) stay bf16.
 - exp() writes fp8e5m2 u = 64*exp(s*score); the 64 cancels in the
   softmax normalization (rowsum trick via a ones-column in V).
 - All partition broadcasts (softmax 1/rowsum, LN mu/rstd, gate) use
   gpsimd partition_broadcast instead of ones-matmuls.
 - LayerNorm interior math runs in bf16 on DVE (2-byte fast modes);
   PSUM evictions and casts are spread across DVE / Pool / ACT.
"""

import math
import numpy as np
from contextlib import ExitStack

import concourse.bacc as bacc
import concourse.bass as bass
import concourse.mybir as mybir
import concourse.tile as tile
from concourse.bass_utils import run_bass_kernel_spmd

dt = mybir.dt
AF = mybir.ActivationFunctionType
ALU = mybir.AluOpType
PM = mybir.MatmulPerfMode
BF16 = dt.np(dt.bfloat16)
F8NP = dt.np(dt.float8e4)

B, S, E, NH, D = 4, 1024, 512, 8, 64
HID = 4 * E
P = 128
KS = E // P          # 4 feature slabs of 128
NT = S // 512        # 2 token n-tiles of 512
MT = S // P          # 8 token m-tiles of 128
HKS = HID // P       # 16 hidden slabs
EPS = 1e-5
WS = 16.0            # host-side fp8 weight scale
C_EXP = 16.0         # softmax exp output scale (cancels in normalization)
LN_C = math.log(C_EXP)
VB = NH * 65 + 8     # v block stride per k-tile, padded to 528:
                     # dual-fp8 Ldweights needs pair stride % 16 == 0

F32 = dt.float32
BF = dt.bfloat16
F8 = dt.float8e4
F8U = dt.float8e5


def _build_nc(scale: float):
    nc = bacc.Bacc("TRN2", target_bir_lowering=False, debug=False,
                   num_devices=8)

    def din(name, shape, dty=F32):
        return nc.dram_tensor(name, shape, dty, kind="ExternalInput").ap()

    # activations (pre-transposed on host, feature-major [E, S])
    xo32 = din("xo32", [E, S])          # own stream input, fp32 (residual)
    xt16 = din("xt16", [E, S], BF)      # other stream input, bf16 (residual)
    xo8 = din("xo8", [E, S], F8)        # own, fp8 (matmul rhs)
    xt8 = din("xt8", [E, S], F8)

    # attention weight sets: a = self-own, b = self-other, c = cross
    # q/k/v fp8 (x16); out-proj bf16 (unscaled) for accuracy
    attw = {}
    for tag in ("a", "b", "c"):
        for m in ("qw", "kw", "vw"):
            attw[tag + m] = din(tag + m, [E, E], F8)
        attw[tag + "ow"] = din(tag + "ow", [E, E], BF)
        attw[tag + "qb"] = din(tag + "qb", [E])     # 16*qb
        attw[tag + "ob"] = din(tag + "ob", [E])     # ob + vb@ow (unscaled)

    f1w = din("f1w", [E, HID], BF)
    f1b = din("f1b", [HID])
    f2w = din("f2w", [HID, E], BF)
    f2b = din("f2b", [E])
    gw = din("gw", [2 * E, 2], F8)
    gbd = din("gbd", [1, 1])            # gate_b[0] - gate_b[1]

    # norm params: a_own, a_oth (post-self-attn), b (post-gate), c (post-ffn)
    nrm = {}
    for tag in ("nao", "nat", "nb", "nc"):
        nrm[tag + "g"] = din(tag + "g", [E])
        nrm[tag + "b"] = din(tag + "b", [E])

    out_t = nc.dram_tensor("outT", [E, S], F32, kind="ExternalOutput").ap()

    with TileKernel(nc, scale) as tk:
        tk.run(xo32, xt16, xo8, xt8, attw, f1w, f1b, f2w, f2b, gw, gbd,
               nrm, out_t)

    nc.finalize()
    return nc


DEBUG_DUMPS = False


class TileKernel:
    def __init__(self, nc, scale):
        self.nc = nc
        self.scale = float(scale)
        self.ctx = ExitStack()
        self.poolid = 0

    attn_idx = 0

    def dump(self, name, ap):
        if not DEBUG_DUMPS:
            return
        d = self.nc.dram_tensor("dbg_" + name, list(ap.shape), ap.dtype,
                                kind="ExternalOutput").ap()
        self.nc.sync.dma_start(d, ap)

    def __enter__(self):
        self.tc = self.ctx.enter_context(tile.TileContext(self.nc))
        return self

    def __exit__(self, *a):
        return self.ctx.__exit__(*a)

    # ---------- helpers ----------

    def load_vec(self, pool, dram_ap, n, name=None):
        """Load a [n*128] fp32 vector as [128, n] (slab per column)."""
        t = pool.tile([P, n], F32, tag=name)
        self.nc.sync.dma_start(
            t[:], dram_ap.rearrange("(s p) -> p s", p=P))
        return t

    def load_w8(self, pool, dram_ap, in_dim, out_dim, name=None, dty=F8):
        """Load weight [in,out] as [128, (in/128)*out] slab-major."""
        ks = in_dim // P
        t = pool.tile([P, ks * out_dim], dty, tag=name)
        self.nc.sync.dma_start(
            t[:].rearrange("p (s o) -> p s o", s=ks),
            dram_ap.rearrange("(s p) o -> p s o", p=P))
        return t

    def proj_bf(self, wsb, rhs16, in_dim, out_dim, evict, bufs=4):
        """bf16 Form-B projection: out^T = W^T @ x^T."""
        nc = self.nc
        self.poolid += 1
        with self.tc.tile_pool(name=f"pb{self.poolid}", bufs=bufs,
                               space="PSUM") as pp:
            nks = in_dim // P
            wr = wsb[:].rearrange("p (s o) -> p s o", s=nks)
            xr = rhs16[:].rearrange("p (s t) -> p s t", s=nks)
            for ms in range(out_dim // P):
                for nt in range(NT):
                    ps = pp.tile([P, 512], F32, tag="proj", name="proj")
                    for k in range(nks):
                        nc.tensor.matmul(
                            ps[:], wr[:, k, ms * P: ms * P + P],
                            xr[:, k, nt * 512: nt * 512 + 512],
                            start=(k == 0), stop=(k == nks - 1))
                    evict(ps, ms, nt)

    def proj_f8(self, wsb, rhs8, in_dim, out_dim, evict, bufs=4):
        """out^T[out,tok] = (W^T @ x^T) with fp8 DoubleRow matmuls.
        evict(ps, ms, nt) consumes a [128,512] fp32 PSUM tile."""
        nc = self.nc
        self.poolid += 1
        with self.tc.tile_pool(name=f"pp{self.poolid}", bufs=bufs,
                               space="PSUM") as pp:
            self._proj_f8(pp, wsb, rhs8, in_dim, out_dim, evict)

    def _proj_f8(self, pp, wsb, rhs8, in_dim, out_dim, evict):
        nc = self.nc
        nks = in_dim // P
        npr = nks // 2
        wr = wsb[:].rearrange("p (s o) -> p s o", s=nks)
        xr = rhs8[:].rearrange("p (s t) -> p s t", s=nks)
        for ms in range(out_dim // P):
            for nt in range(NT):
                ps = pp.tile([P, 512], F32, tag="proj", name="proj")
                for win in range(2):
                    o = ps[:, win * 256: win * 256 + 256]
                    toff = nt * 512 + win * 256
                    for kp in range(npr):
                        nc.tensor.matmul(
                            o,
                            wr[:, 2 * kp: 2 * kp + 2, ms * P: ms * P + P],
                            xr[:, 2 * kp: 2 * kp + 2, toff: toff + 256],
                            start=(kp == 0), stop=(kp == npr - 1),
                            perf_mode=PM.DoubleRow)
                evict(ps, ms, nt)

    # ---------- attention ----------

    def attention_core(self, name, q8, kv8, wq, wk, wv, qb, oup):
        """MHA core: fp8 projections, bf16 scores, fp8 exp, DoubleRow AV,
        streamed softmax normalization.  Returns the normalized per-head
        output o16 (bf16, tile in caller pool `oup`, = 16x true o).
        The caller runs the out-projection separately (attention_finish)
        so the next attention's core can overlap this one's tail."""
        nc, tc = self.nc, self.tc
        ou = oup.tile([P, KS * S], BF, tag="ou", name="ou_" + name,
                      bufs=2)
        with ExitStack() as actx:
            ap = actx.enter_context(
                tc.tile_pool(name="attc_" + name, bufs=1))
            up = actx.enter_context(
                tc.tile_pool(name="attu_" + name, bufs=3))

            qt = ap.tile([P, KS * S], BF, tag="qT")
            kt = ap.tile([P, KS * S], BF, tag="kT")
            vt = ap.tile([P, MT * VB], F8, tag="vT")

            def ev_q(ps, ms, nt):
                nc.vector.tensor_scalar(
                    qt[:, ms * S + nt * 512: ms * S + nt * 512 + 512],
                    ps[:], qb[:, ms: ms + 1], None, op0=ALU.add)

            def ev_k(ps, ms, nt):
                # ACT (idle during projections; GPSIMD cannot read PSUM)
                nc.scalar.copy(
                    kt[:, ms * S + nt * 512: ms * S + nt * 512 + 512], ps[:])

            self.poolid += 1
            with tc.tile_pool(name=f"attn_pp{self.poolid}", bufs=4,
                              space="PSUM") as pp:
                self._proj_f8(pp, wq, q8, E, E, ev_q)
                self._proj_f8(pp, wk, kv8, E, E, ev_k)

                # V: Form A (x^T as lhsT) -> token-major v [tok, feat],
                # strided into per-head 65-wide blocks, col 64 = 1.
                v4 = vt[:].rearrange("p (m c) -> p m c", m=MT)
                nc.gpsimd.memset(
                    v4[:, :, 0:NH * 65]
                    .rearrange("p m (h c) -> p m h c", h=NH)
                    [:, :, :, 64:65], 1.0)
                xr = kv8[:].rearrange("p (s t) -> p s t", s=KS)
                wvr = wv[:].rearrange("p (s o) -> p s o", s=KS)
                for mt in range(MT):
                    ps = pp.tile([P, 512], F32, tag="proj", name="proj")
                    for fw in range(2):
                        o = ps[:, fw * 256: fw * 256 + 256]
                        for kp in range(2):
                            nc.tensor.matmul(
                                o,
                                xr[:, 2 * kp: 2 * kp + 2, mt * P: mt * P + P],
                                wvr[:, 2 * kp: 2 * kp + 2,
                                    fw * 256: fw * 256 + 256],
                                start=(kp == 0), stop=(kp == 1),
                                perf_mode=PM.DoubleRow)
                    nc.scalar.copy(
                        v4[:, mt, 0:NH * 65]
                        .rearrange("p (h c) -> p h c", h=NH)[:, :, 0:64],
                        ps[:].rearrange("p (h d) -> p h d", h=NH))

            self.poolid += 1
            scav = ExitStack()
            sp = scav.enter_context(
                tc.tile_pool(name=f"attn_sc{self.poolid}", bufs=2,
                             space="PSUM"))
            avp = scav.enter_context(
                tc.tile_pool(name=f"attn_av{self.poolid}", bufs=4,
                             space="PSUM"))

            if name == "own":
                self.dump("o_qt", qt[:])
                self.dump("o_kt", kt[:])
                self.dump("o_vt", vt[:])
            v4 = vt[:].rearrange("p (m c) -> p m c", m=MT)
            sexp = self.scale / (WS * WS)
            # head pairs outer, qn inner: each pair's softmax rowsums are
            # reciprocal'd + broadcast + applied as soon as the pair is
            # done, overlapping the remaining pairs' scores/exp/AV.
            for j in range(NH // 2):
                hs = (2 * j, 2 * j + 1)
                rbh = {}
                for qn in range(NT):
                    av = {(h, w): avp.tile([65, 256], F32, tag="av",
                                           name=f"av{h}_{w}")
                          for h in hs for w in range(2)}
                    for wave in range(MT // 2):
                        sc = {h: sp.tile([P, 1024], F32, tag="sc",
                                         name=f"sc{h}") for h in hs}
                        for i in range(2):
                            mt = wave * 2 + i
                            for h in hs:
                                bp = (h % 2) * 64
                                sl = h // 2
                                nc.tensor.matmul(
                                    sc[h][:, i * 512: i * 512 + 512],
                                    kt[bp: bp + 64,
                                       sl * S + mt * P: sl * S + mt * P + P],
                                    qt[bp: bp + 64,
                                       sl * S + qn * 512: sl * S + qn * 512 + 512],
                                    start=True, stop=True)
                        ut = {h: up.tile([P, 1024], F8, tag="u",
                                         name=f"u{h}") for h in hs}
                        for h in hs:
                            nc.scalar.activation(
                                ut[h][:], sc[h][:], AF.Exp, scale=sexp,
                                bias=self.lnc_c[:, 0:1])
                        for h in hs:
                            utr = ut[h][:].rearrange("p (i t) -> p i t", i=2)
                            for win in range(2):
                                nc.tensor.matmul(
                                    av[h, win][:],
                                    v4[:, 2 * wave: 2 * wave + 2,
                                       h * 65: h * 65 + 65],
                                    utr[:, :, win * 256: win * 256 + 256],
                                    start=(wave == 0), stop=(wave == 3),
                                    perf_mode=PM.DoubleRow)
                    for h in hs:
                        bp = (h % 2) * 64
                        sl = h // 2
                        # unnormalized o^T -> SBUF; 1/rowsum -> broadcast
                        qoff = sl * S + qn * 512
                        st = up.tile([1, 512], F32, tag="rstage",
                                     name="rstage")
                        for win in range(2):
                            nc.vector.tensor_copy(
                                ou[bp: bp + 64,
                                   qoff + win * 256: qoff + win * 256 + 256],
                                av[h, win][0:64, :])
                            nc.vector.tensor_copy(
                                st[:, win * 256: win * 256 + 256],
                                av[h, win][64:65, :])
                        rr = up.tile([1, 512], BF, tag="rrec",
                                     name="rrec")
                        with nc.allow_low_precision(
                                reason="1/rowsum to bf16 is plenty"):
                            nc.vector.reciprocal(rr[:], st[:])
                        # full-128 broadcast: HW ucode mishandles
                        # non-zero destination base partitions
                        rbt = ap.tile([P, 512], BF, tag="rbh",
                                      name=f"rbh{h % 2}_{qn}", bufs=6)
                        nc.gpsimd.partition_broadcast(rbt[:], rr[:])
                        rbh[h, qn] = rbt
                for h in hs:
                    bp = (h % 2) * 64
                    sl = h // 2
                    for qn in range(NT):
                        qoff = sl * S + qn * 512
                        nc.vector.tensor_tensor(
                            ou[bp: bp + 64, qoff: qoff + 512],
                            ou[bp: bp + 64, qoff: qoff + 512],
                            rbh[h, qn][bp: bp + 64, :], op=ALU.mult)
                if name == "own" and j == 0:
                    self.dump("o_rb0", rbh[0, 0][:])
            scav.close()
        if name == "own":
            self.dump("o_ou", ou[:])
        return ou

    def attention_finish(self, ou, wo, evict_out, bufs=2):
        """Out projection (bf16) of a finished attention core.  bufs=2
        keeps PSUM pressure low enough to coexist with the next
        attention core's score/AV pools; the cross finish (no
        concurrent core) can pipeline deeper."""
        self.proj_bf(wo, ou, E, E, evict_out, bufs=bufs)

    # ---------- layernorm ----------

    def layer_norm(self, t32, gam, bet, out32, out8, out16=None,
                   out_dma=None, bf_in=False):
        """LN over features (partition axis) of t32 [128, KS*S].
        Stats come from a bf16 copy (ones-matmul over partitions); the
        normalize path runs in fp32 when out32 is requested (accuracy),
        bf16 otherwise.  Optional fp8 / bf16 side outputs; out_dma
        streams the fp32 output to DRAM per slab.  bf_in: t32 is
        already bf16 (skip the cast)."""
        nc = self.nc
        self.poolid += 1
        with self.tc.tile_pool(name=f"lnsb{self.poolid}", bufs=1) as lnp:
            if bf_in:
                t16 = t32
            else:
                t16 = lnp.tile([P, KS * S], BF, tag="ln_t16")
                for k in range(KS):
                    sl = slice(k * S, k * S + S)
                    nc.gpsimd.tensor_copy(t16[:, sl], t32[:, sl])
            mu = lnp.tile([1, S], F32, tag="ln_mu", name="ln_mu")
            var = lnp.tile([1, S], F32, tag="ln_row", name="ln_var",
                           bufs=2)
            self.poolid += 1
            with self.tc.tile_pool(name=f"lnp{self.poolid}", bufs=2,
                                   space="PSUM") as sp1:
                for nt in range(NT):
                    pmu = sp1.tile([1, 512], F32, tag="ln_stat", name="pmu")
                    psq = sp1.tile([1, 512], F32, tag="ln_stat", name="psq")
                    for k in range(KS):
                        sl = slice(k * S + nt * 512, k * S + nt * 512 + 512)
                        tsq = lnp.tile([P, 512], BF, tag="ln_tsq",
                                       name="ln_tsq", bufs=2)
                        nc.vector.tensor_tensor(tsq[:], t16[:, sl],
                                                t16[:, sl], op=ALU.mult)
                        nc.tensor.matmul(
                            pmu[:], self.ones_mean[:, 0:1], t16[:, sl],
                            start=(k == 0), stop=(k == KS - 1))
                        nc.tensor.matmul(
                            psq[:], self.ones_mean[:, 0:1], tsq[:],
                            start=(k == 0), stop=(k == KS - 1))
                    osl = slice(nt * 512, nt * 512 + 512)
                    nc.vector.tensor_copy(mu[:, osl], pmu[:])
                    mu2 = lnp.tile([1, 512], F32, tag="ln_mu2", name="ln_mu2")
                    nc.vector.tensor_tensor(mu2[:], mu[:, osl], mu[:, osl],
                                            op=ALU.mult)
                    nc.vector.tensor_tensor(var[:, osl], psq[:], mu2[:],
                                            op=ALU.subtract)
            # rstd = exp(-0.5*ln(var+eps)) (tiny rows)
            lnv = lnp.tile([1, S], F32, tag="ln_row", name="ln_lnv",
                           bufs=2)
            nc.scalar.activation(lnv[:], var[:], AF.Ln,
                                 bias=self.eps_c[:, 0:1])
            rstd = lnp.tile([1, S], F32, tag="ln_row", name="ln_rstd",
                            bufs=2)
            nc.scalar.activation(rstd[:], lnv[:], AF.Exp, scale=-0.5)
            fp32_path = out32 is not None
            bdt = F32 if fp32_path else BF
            if fp32_path:
                murow, rsrow = mu, rstd
            else:
                murow = lnp.tile([1, S], BF, tag="ln_mu16")
                nc.vector.tensor_copy(murow[:], mu[:])
                rsrow = lnp.tile([1, S], BF, tag="ln_rstd16")
                nc.vector.tensor_copy(rsrow[:], rstd[:])
            mub = lnp.tile([P, S], bdt, tag="ln_mub")
            rstdb = lnp.tile([P, S], bdt, tag="ln_rstdb")
            nc.gpsimd.partition_broadcast(mub[:], murow[:])
            nc.gpsimd.partition_broadcast(rstdb[:], rsrow[:])
            src = t32 if fp32_path else t16
            for k in range(KS):
                for nh in range(NT):
                    sl = slice(k * S + nh * 512, k * S + nh * 512 + 512)
                    bsl = slice(nh * 512, nh * 512 + 512)
                    w = lnp.tile([P, 512], bdt, tag="ln_w", name="ln_w",
                                 bufs=2)
                    nc.vector.tensor_tensor(w[:], src[:, sl], mub[:, bsl],
                                            op=ALU.subtract)
                    nc.vector.tensor_tensor(w[:], w[:], rstdb[:, bsl],
                                            op=ALU.mult)
                    if out32 is not None:
                        nc.vector.tensor_scalar(
                            out32[:, sl], w[:], gam[:, k: k + 1],
                            bet[:, k: k + 1], op0=ALU.mult, op1=ALU.add)
                    if out16 is not None:
                        nc.gpsimd.tensor_scalar(
                            out16[:, sl], w[:], gam[:, k: k + 1],
                            bet[:, k: k + 1], op0=ALU.mult, op1=ALU.add)
                    if out8 is not None:
                        eng = nc.gpsimd if out16 is None else nc.vector
                        eng.tensor_scalar(
                            out8[:, sl], w[:], gam[:, k: k + 1],
                            bet[:, k: k + 1], op0=ALU.mult, op1=ALU.add)
                if out32 is not None and out_dma is not None:
                    nc.sync.dma_start(
                        out_dma.rearrange("(s p) t -> p s t", p=P)
                        [:, k, :],
                        out32[:, k * S: k * S + S])

    # ---------- main ----------

    def run(self, xo32, xt16, xo8, xt8, attw, f1w, f1b, f2w, f2b, gw, gbd,
            nrm, out_t):
        nc, tc, ctx = self.nc, self.tc, self.ctx

        const = ctx.enter_context(tc.tile_pool(name="const", bufs=1))

        self.ones_mean = const.tile([P, 1], BF)
        nc.vector.memset(self.ones_mean[:], 1.0 / E)
        self.eps_c = const.tile([1, 1], F32)
        nc.vector.memset(self.eps_c[:], EPS)
        self.lnc_c = const.tile([P, 1], F32)
        nc.vector.memset(self.lnc_c[:], LN_C)
        # ---- weight prefetch: set 'a' first, then the stage-1 inputs
        # (unblocking the first projections ASAP), then the rest ----
        wp = ctx.enter_context(tc.tile_pool(name="wp_all", bufs=1))
        act = ctx.enter_context(tc.tile_pool(name="acts", bufs=1))
        oup = ctx.enter_context(tc.tile_pool(name="oup", bufs=1))

        W = {}

        def load_set(tag):
            for m in ("qw", "kw", "vw"):
                W[tag + m] = self.load_w8(wp, attw[tag + m], E, E, tag + m)
            W[tag + "ow"] = self.load_w8(wp, attw[tag + "ow"], E, E,
                                         tag + "ow", dty=BF)
            W[tag + "qb"] = self.load_vec(wp, attw[tag + "qb"], KS,
                                          tag + "qb")
            W[tag + "ob"] = self.load_vec(wp, attw[tag + "ob"], KS,
                                          tag + "ob")

        load_set("a")

        # ---- stage 1: self-attention + LN for both streams ----
        # Emission order: own.core, oth.core, own.finish, oth.finish —
        # the oth core's ACT-bound exp phase overlaps own's DVE/Pool
        # finish (out-proj evictions + LN).
        s1 = ExitStack()
        pools = {st: s1.enter_context(tc.tile_pool(name="sb_" + st,
                                                   bufs=1))
                 for st in ("own", "oth")}
        s1x = ExitStack()
        x8p = s1x.enter_context(tc.tile_pool(name="s1x", bufs=1))
        xin = {}
        for st, (x32d, x8d) in (("own", (xo32, xo8)),
                                ("oth", (xt16, xt8))):
            sbp = pools[st]
            x8 = x8p.tile([P, KS * S], F8, tag="x8", name="x8" + st,
                          bufs=2)
            nc.sync.dma_start(
                x8[:].rearrange("p (s t) -> p s t", s=KS),
                x8d.rearrange("(s p) t -> p s t", p=P))
            xdt = F32 if st == "own" else BF
            x32 = sbp.tile([P, KS * S], xdt, tag="x32", name="x32")
            nc.sync.dma_start(
                x32[:].rearrange("p (s t) -> p s t", s=KS),
                x32d.rearrange("(s p) t -> p s t", p=P))
            xin[st] = (sbp, x8, x32)

        load_set("b")
        load_set("c")
        gw_sb = wp.tile([P, 8 * 2], F8, tag="gw")
        nc.sync.dma_start(
            gw_sb[:].rearrange("p (s o) -> p s o", s=8),
            gw.rearrange("(s p) o -> p s o", p=P))
        # norm params / gate consts aren't needed until the first LN
        # (~150us in) — load them after the startup-critical DMAs
        self.gbdneg = const.tile([1, 1], F32)
        nc.sync.dma_start(self.gbdneg[:], gbd[:])
        nc.vector.tensor_scalar(self.gbdneg[:], self.gbdneg[:], -1.0, None,
                                op0=ALU.mult)
        gam = {t: self.load_vec(const, nrm[t + "g"], KS, name=t + "g")
               for t in ("nao", "nat", "nb", "nc")}
        bet = {t: self.load_vec(const, nrm[t + "b"], KS, name=t + "b")
               for t in ("nao", "nat", "nb", "nc")}

        ou1 = {}
        for st, wtag in (("own", "a"), ("oth", "b")):
            sbp, x8, x32 = xin[st]
            ou1[st] = self.attention_core(
                st, x8, x8, W[wtag + "qw"], W[wtag + "kw"],
                W[wtag + "vw"], W[wtag + "qb"], oup)
        s1x.close()

        y32 = None
        y8 = {}
        for st, (wtag, ntag) in (("own", ("a", "nao")),
                                 ("oth", ("b", "nat"))):
            sbp, x8, x32 = xin[st]
            ob = W[wtag + "ob"]
            t1 = x32  # residual accumulates in place over the input
            # residual + ob are pre-scaled x16 on the host; psum is
            # 16*(o@ow), so t1 = 16*(true t1).  LN is scale-invariant.

            def ev_out(ps, ms, nt, _ob=ob, _t1=t1):
                sl = slice(ms * S + nt * 512, ms * S + nt * 512 + 512)
                nc.vector.scalar_tensor_tensor(
                    _t1[:, sl], ps[:], _ob[:, ms: ms + 1], _t1[:, sl],
                    op0=ALU.add, op1=ALU.add)

            self.attention_finish(ou1[st], W[wtag + "ow"], ev_out)
            if st == "own":
                self.dump("t1own", t1[:])
                y32 = act.tile([P, KS * S], F32, tag="a32",
                               name="yo32", bufs=2)
                y8[st] = act.tile([P, KS * S], F8, tag="a8",
                                  name="yo8", bufs=3)
                self.layer_norm(t1, gam[ntag], bet[ntag], y32, y8[st])
                self.dump("y32", y32[:])
            else:
                y8[st] = act.tile([P, KS * S], F8, tag="a8",
                                  name="yt8", bufs=3)
                self.layer_norm(t1, gam[ntag], bet[ntag], None, y8[st],
                                bf_in=True)
        s1.close()

        # ---- stage 2: cross attention ----
        # FFN weights load here: early enough to overlap, after the
        # stage-1 SBUF peak has passed.
        wpf = ctx.enter_context(tc.tile_pool(name="wp_ffn", bufs=1))
        w1 = self.load_w8(wpf, f1w, E, HID, "w1", dty=BF)
        b1 = self.load_vec(wpf, f1b, HKS, "b1")
        w2 = self.load_w8(wpf, f2w, HID, E, "w2", dty=BF)
        b2 = self.load_vec(wpf, f2b, KS, "b2")

        cross32 = act.tile([P, KS * S], F32, tag="a32", bufs=2)
        cross8 = act.tile([P, KS * S], F8, tag="a8", bufs=3)
        with ExitStack() as sctx:
            sbp = sctx.enter_context(tc.tile_pool(name="sb_c", bufs=1))
            ob = W["cob"]

            ouc = self.attention_core(
                "cross", y8["own"], y8["oth"], W["cqw"], W["ckw"],
                W["cvw"], W["cqb"], oup)

            def ev_cross(ps, ms, nt, _ob=ob):
                sl = slice(ms * S + nt * 512, ms * S + nt * 512 + 512)
                nc.vector.tensor_scalar(
                    cross32[:, sl], ps[:], 1.0 / WS,
                    _ob[:, ms: ms + 1], op0=ALU.mult, op1=ALU.add)
                nc.gpsimd.tensor_copy(cross8[:, sl], cross32[:, sl])

            self.attention_finish(ouc, W["cow"], ev_cross, bufs=4)
            self.dump("cross32", cross32[:])

        # ---- stage 3: gate + merge + LN_b ----
        with ExitStack() as sctx:
            sbp = sctx.enter_context(tc.tile_pool(name="sb_g", bufs=1))
            g0row = sbp.tile([1, S], F32, tag="g0")
            gwr = gw_sb[:].rearrange("p (s o) -> p s o", s=8)
            self.poolid += 1
            gp = sctx.enter_context(tc.tile_pool(
                name=f"gp{self.poolid}", bufs=2, space="PSUM"))
            srcs = (y8["own"], cross8)
            for nt in range(NT):
                l0 = gp.tile([1, 512], F32, tag="gl", name="gl0")
                l1 = gp.tile([1, 512], F32, tag="gl", name="gl1")
                for s in range(8):  # 8 gw slabs: 0-3 own, 4-7 cross
                    src = srcs[s // 4]
                    xr = src[:].rearrange("p (s t) -> p s t", s=KS)
                    for col, l in ((0, l0), (1, l1)):
                        nc.tensor.matmul(
                            l[:], gwr[:, s, col: col + 1],
                            xr[:, s % 4, nt * 512: nt * 512 + 512],
                            start=(s == 0), stop=(s == 7))
                l0s = sbp.tile([1, 512], F32, tag="gl0s", name="gl0s")
                nc.scalar.copy(l0s[:], l0[:])
                d = sbp.tile([1, 512], F32, tag="gd", name="gd")
                nc.vector.tensor_tensor(d[:], l1[:], l0s[:],
                                        op=ALU.subtract)
                # g0 = sigmoid(l0-l1+gbd) = 1/(1+exp(l1-l0-gbd))
                eneg = sbp.tile([1, 512], F32, tag="ge", name="ge")
                nc.scalar.activation(eneg[:], d[:], AF.Exp,
                                     scale=1.0 / WS,
                                     bias=self.gbdneg[:, 0:1])
                den = sbp.tile([1, 512], F32, tag="gden", name="gden")
                nc.vector.tensor_scalar(den[:], eneg[:], 1.0, None,
                                        op0=ALU.add)
                nc.vector.reciprocal(
                    g0row[:, nt * 512: nt * 512 + 512], den[:])
            g0b = sbp.tile([P, S], F32, tag="g0b")
            nc.gpsimd.partition_broadcast(g0b[:], g0row[:])
            t2 = sbp.tile([P, KS * S], F32, tag="t2")
            for k in range(KS):
                sl = slice(k * S, k * S + S)
                w = sbp.tile([P, S], F32, tag="gs", name="gs", bufs=2)
                nc.vector.tensor_tensor(w[:], y32[:, sl],
                                        cross32[:, sl], op=ALU.subtract)
                nc.vector.tensor_tensor(w[:], w[:], g0b[:], op=ALU.mult)
                nc.vector.tensor_tensor(t2[:, sl], w[:], cross32[:, sl],
                                        op=ALU.add)
            self.dump("g0row", g0row[:])
            self.dump("t2", t2[:])
            z32 = act.tile([P, KS * S], F32, tag="a32", bufs=2)
            z16 = act.tile([P, KS * S], BF, tag="a16", bufs=1)
            self.layer_norm(t2, gam["nb"], bet["nb"], z32, None,
                            out16=z16)
            self.dump("z32", z32[:])

        # ---- stage 4: FFN (bf16) + LN_c + output ----
        with ExitStack() as sctx:
            sbp = sctx.enter_context(tc.tile_pool(name="sb_f", bufs=1))
            t3 = z32  # FFN residual accumulates in place over z32
            with ExitStack() as fctx:
                hp = fctx.enter_context(tc.tile_pool(name="hp_f", bufs=1))
                h16 = hp.tile([P, HKS * S], BF, tag="h16")

                def ev_gelu(ps, ms, nt):
                    nc.scalar.activation(
                        h16[:, ms * S + nt * 512: ms * S + nt * 512 + 512],
                        ps[:], AF.Gelu, bias=b1[:, ms: ms + 1])

                self.proj_bf(w1, z16, E, HID, ev_gelu)

                def ev_f2(ps, ms, nt):
                    sl = slice(ms * S + nt * 512, ms * S + nt * 512 + 512)
                    nc.vector.scalar_tensor_tensor(
                        t3[:, sl], ps[:], b2[:, ms: ms + 1], z32[:, sl],
                        op0=ALU.add, op1=ALU.add)

                self.proj_bf(w2, h16, HID, E, ev_f2)

            out32 = sbp.tile([P, KS * S], F32, tag="out32")
            self.layer_norm(t3, gam["nc"], bet["nc"], out32, None,
                            out_dma=out_t)


_NC_CACHE = {}


def _get_nc(scale):
    key = round(float(scale), 12)
    if key not in _NC_CACHE:
        _NC_CACHE[key] = _build_nc(scale)
    return _NC_CACHE[key]


def _prep_in_maps(inputs):
    """Slice/transform the full inputs into 8 per-core input dicts."""
    f32 = np.float32
    body = np.asarray(inputs["body_feats"], f32)
    limb = np.asarray(inputs["limb_feats"], f32)
    qw = np.asarray(inputs["attn_qw"], f32)
    qb = np.asarray(inputs["attn_qb"], f32)
    kw = np.asarray(inputs["attn_kw"], f32)
    vw = np.asarray(inputs["attn_vw"], f32)
    vb = np.asarray(inputs["attn_vb"], f32)
    ow = np.asarray(inputs["attn_ow"], f32)
    ob = np.asarray(inputs["attn_ob"], f32)
    f1w = np.asarray(inputs["ffn_w1"], f32)
    f1b = np.asarray(inputs["ffn_b1"], f32)
    f2w = np.asarray(inputs["ffn_w2"], f32)
    f2b = np.asarray(inputs["ffn_b2"], f32)
    ns = np.asarray(inputs["norm_scale"], f32)
    nb = np.asarray(inputs["norm_bias"], f32)
    gw = np.asarray(inputs["gate_w"], f32)
    gb = np.asarray(inputs["gate_b"], f32)

    feats = [body, limb]
    ob_eff = [ob[i] + vb[i] @ ow[i] for i in range(4)]
    gbd = np.array([[gb[0] - gb[1]]], f32)
    ln_a = [0, 3]
    ln_c = [2, 5]

    in_maps = []
    for c in range(8):
        b, s = c // 2, c % 2
        o = s          # own stream / self-attn set
        t = 1 - s      # other stream
        cr = 2 + s     # cross-attn set
        xoT = np.ascontiguousarray(feats[o][b].T)
        xtT = np.ascontiguousarray(feats[t][b].T)
        m = {
            # residual streams pre-scaled x16 (the stage-1 evict adds
            # them to 16x psums; LN is scale-invariant)
            "xo32": WS * xoT,
            "xt16": (WS * xtT).astype(BF16),
            "xo8": xoT.astype(F8NP),
            "xt8": xtT.astype(F8NP),
            "f1w": f1w[s].astype(BF16), "f1b": f1b[s],
            "f2w": f2w[s].astype(BF16), "f2b": f2b[s],
            "gw": (WS * gw).astype(F8NP), "gbd": gbd,
            "naog": ns[ln_a[o]], "naob": nb[ln_a[o]],
            "natg": ns[ln_a[t]], "natb": nb[ln_a[t]],
            "nbg": ns[1], "nbb": nb[1],
            "ncg": ns[ln_c[s]], "ncb": nb[ln_c[s]],
        }
        for tag, i in (("a", o), ("b", t), ("c", cr)):
            m[tag + "qw"] = (WS * qw[i]).astype(F8NP)
            m[tag + "kw"] = (WS * kw[i]).astype(F8NP)
            m[tag + "vw"] = (WS * vw[i]).astype(F8NP)
            m[tag + "ow"] = ow[i].astype(BF16)
            m[tag + "qb"] = WS * qb[i]
            # self-attn evicts add ob to a 16x psum; cross runs at 1x
            m[tag + "ob"] = (WS if tag != "c" else 1.0) * ob_eff[i]
        in_maps.append(m)
    return in_maps


def kernel(**inputs):
    temp = float(np.asarray(inputs["temperature"]))
    scale = (D ** -0.5) / temp
    nc = _get_nc(scale)
    in_maps = _prep_in_maps(inputs)
    res = run_bass_kernel_spmd(nc, in_maps, core_ids=list(range(8)))
    body = np.empty((B, S, E), np.float32)
    limb = np.empty((B, S, E), np.float32)
    for c in range(8):
        b, s = c // 2, c % 2
        o = res.results[c]["outT"].T
        (body if s == 0 else limb)[b] = o
    return body, limb
